# revision 7
# baseline (speedup 1.0000x reference)
"""HGCN Bass/Trainium2 kernel — 8-core SPMD, data-parallel over batch N.

Strategy:
  - Shard batch N=16 over 8 cores (2 per core); replicate adjacencies + weights.
  - Per-core layout: j = (n, l, c) flattened (row = n*384 + l*32 + c), J = 768.
    Fine diffusion runs as out[j, w] = sum_v xj[v, j] * A[v, w] with xj resident
    in SBUF as the stationary operand (fp32r) and support streamed from HBM.
  - 1x1 conv (contraction over channels c, which live inside the partition dim)
    uses 4-group diagonal tile_position matmuls with a stacked weight tile.
  - The super-adjacency as_mat couples all batches (mismatched torch flatten
    orders), so sxg is AllGathered (196KB/rank) and as_mat computed redundantly
    per core; the right operand's row permutation is applied with an
    indirect-DMA gather using host-precomputed indices.
  - Hierarchical relu fusions: afc/acs contractions via PE with on-the-fly PE
    transposes; step-5 (afc.T @ hf) accumulates into a persistent 3-bank PSUM
    tensor while hf tiles stream out.
"""
import os
import sys

sys.path.insert(0, "/opt/trn_rl_repo")

import numpy as np
from contextlib import ExitStack

import concourse.bass as bass
import concourse.tile as tile
from concourse import bacc, mybir
from concourse.bass_utils import run_bass_kernel_spmd
from concourse.masks import make_identity

dt = mybir.dt
F32 = dt.float32
F32R = dt.float32r

# dims
N, C, V, VC, S, L, CO = 16, 32, 2048, 256, 64, 12, 32
N1, N2, N3, N4 = 0.8, 0.2, 0.2, 0.2
NCORES = 8
NLOC = N // NCORES            # 2 batches per core
J = NLOC * L * C              # 768 local (n,l,c) columns
JT = J // 128                 # 6 j-tiles
VT = V // 128                 # 16 v-tiles
WCH = 256                     # fine w chunk (free dim)
NW = V // WCH                 # 8 w chunks
WCT = VC // 128               # 2 coarse-node tiles
AT = (NCORES * J) // 128      # 48 as_mat contraction tiles

_CACHE = {}


def _build():
    nc = bacc.Bacc("TRN2", target_bir_lowering=False, debug=False,
                   num_devices=NCORES)

    # ---- I/O ----
    xj_d = nc.declare_dram_parameter("xj", [V, J], F32, isOutput=False)
    xt_d = nc.declare_dram_parameter("xt", [J, V], F32, isOutput=False)
    a0_d = nc.declare_dram_parameter("a0", [V, V], F32, isOutput=False)
    a1_d = nc.declare_dram_parameter("a1", [V, V], F32, isOutput=False)
    c0_d = nc.declare_dram_parameter("c0", [VC, VC], F32, isOutput=False)
    c1_d = nc.declare_dram_parameter("c1", [VC, VC], F32, isOutput=False)
    afc_d = nc.declare_dram_parameter("afc", [V, VC], F32, isOutput=False)
    afct_d = nc.declare_dram_parameter("afct", [VC, V], F32, isOutput=False)
    acs_d = nc.declare_dram_parameter("acs", [VC, S], F32, isOutput=False)
    acst_d = nc.declare_dram_parameter("acst", [S, VC], F32, isOutput=False)
    ws_d = nc.declare_dram_parameter("wstack", [128, 96], F32, isOutput=False)
    b128_d = nc.declare_dram_parameter("b128", [128, 1], F32, isOutput=False)
    a2i_d = nc.declare_dram_parameter("a2idx", [128, AT], dt.int32, isOutput=False)

    debug = bool(os.environ.get("HGCN_DEBUG"))
    hf_d = nc.declare_dram_parameter("hf", [J, V], F32, isOutput=True)
    hc_d = nc.declare_dram_parameter("hc", [J, VC], F32, isOutput=True)
    hs_d = nc.declare_dram_parameter("hs", [J, S], F32, isOutput=True)

    if debug:
        dbg_asm_d = nc.declare_dram_parameter("dbg_asm", [S, 2 * S], F32, isOutput=True)
        dbg_hc1_d = nc.declare_dram_parameter("dbg_hc1", [J, VC], F32, isOutput=True)
        dbg_hspre_d = nc.declare_dram_parameter("dbg_hspre", [J, S], F32, isOutput=True)
        dbg_t5_d = nc.declare_dram_parameter("dbg_t5", [J, VC], F32, isOutput=True)
        dbg_sxg_d = nc.declare_dram_parameter("dbg_sxg", [J, S], F32, isOutput=True)
    cc_in = nc.dram_tensor("cc_in", [J, S], F32)
    cc_out = nc.dram_tensor("cc_out", [NCORES * J, S], F32, addr_space="Shared")

    def r(ap):
        return ap.bitcast(F32R)

    with tile.TileContext(nc) as tc:
        with ExitStack() as ctx:
            # ---- pools ----
            const = ctx.enter_context(tc.tile_pool(name="const", bufs=1))
            bigsc = ctx.enter_context(tc.tile_pool(name="bigsc", bufs=3))
            small = ctx.enter_context(tc.tile_pool(name="small", bufs=1))
            astr = ctx.enter_context(tc.tile_pool(name="astr", bufs=34))
            xtstr = ctx.enter_context(tc.tile_pool(name="xtstr", bufs=6))
            afctstr = ctx.enter_context(tc.tile_pool(name="afctstr", bufs=4))
            workf = ctx.enter_context(tc.tile_pool(name="workf", bufs=4))
            works = ctx.enter_context(tc.tile_pool(name="works", bufs=2))
            ys = ctx.enter_context(tc.tile_pool(name="ys", bufs=13))
            agp = ctx.enter_context(tc.tile_pool(name="agp", bufs=8))

            pd = ctx.enter_context(tc.tile_pool(name="pd", bufs=2, space="PSUM"))
            pcf = ctx.enter_context(tc.tile_pool(name="pcf", bufs=2, space="PSUM"))
            ptr = ctx.enter_context(tc.tile_pool(name="ptr", bufs=2, space="PSUM"))
            p5p = ctx.enter_context(tc.tile_pool(name="p5p", bufs=2, space="PSUM"))

            # ---- constants / resident loads ----
            ident = const.tile([128, 128], F32, tag="ident")
            make_identity(nc, ident[:])

            xj_sb = const.tile([128, VT, J], F32R, tag="xj")
            for vt in range(VT):
                nc.sync.dma_start(xj_sb[:, vt, :], r(xj_d.ap()[vt * 128:(vt + 1) * 128, :]))

            afc_sb = const.tile([128, VT, VC], F32R, tag="afc")
            for vt in range(VT):
                nc.sync.dma_start(afc_sb[:, vt, :], r(afc_d.ap()[vt * 128:(vt + 1) * 128, :]))

            acs_r = const.tile([128, WCT, S], F32R, tag="acs_r")
            acs32 = const.tile([128, WCT, S], F32, tag="acs32")
            for wct in range(WCT):
                nc.sync.dma_start(acs_r[:, wct, :], r(acs_d.ap()[wct * 128:(wct + 1) * 128, :]))
                nc.sync.dma_start(acs32[:, wct, :], acs_d.ap()[wct * 128:(wct + 1) * 128, :])
            acst32 = const.tile([S, VC], F32, tag="acst32")
            nc.sync.dma_start(acst32[:], acst_d.ap())

            supc = const.tile([128, 2, WCT, VC], F32R, tag="supc")
            for k, cd in enumerate((c0_d, c1_d)):
                for wct in range(WCT):
                    nc.sync.dma_start(supc[:, k, wct, :], r(cd.ap()[wct * 128:(wct + 1) * 128, :]))

            wsbf = const.tile([128, 96], dt.bfloat16, tag="wsbf")
            ws_f32tmp = const.tile([128, 96], F32, tag="ws_f32tmp")
            nc.sync.dma_start(ws_f32tmp[:], ws_d.ap())
            nc.vector.tensor_copy(wsbf[:], ws_f32tmp[:])
            ws32 = const.tile([128, 96], F32, tag="ws32")
            nc.sync.dma_start(ws32[:], ws_d.ap())
            b128 = const.tile([128, 1], F32, tag="b128")
            nc.sync.dma_start(b128[:], b128_d.ap())
            a2i_sb = const.tile([128, AT], dt.int32, tag="a2i")
            nc.sync.dma_start(a2i_sb[:], a2i_d.ap())
            neg_half = const.tile([128, 1], F32, tag="neg_half")
            nc.gpsimd.memset(neg_half[:], -0.5)
            sc_n1 = const.tile([128, 1], F32, tag="sc_n1")
            nc.gpsimd.memset(sc_n1[:], N1)
            sc_02 = const.tile([128, 1], F32, tag="sc_02")
            nc.gpsimd.memset(sc_02[:], N2)

            # ---- phase A: xc, xcT, sxg, sxgT ----
            xc_sb = bigsc.tile([128, JT, VC], F32, tag="bigsc")
            for jt in range(JT):
                ps = pd.tile([128, VC], F32, tag="pd")
                for vt in range(VT):
                    nc.tensor.matmul(ps[:], xj_sb[:, vt, jt * 128:(jt + 1) * 128],
                                     afc_sb[:, vt, :],
                                     start=(vt == 0), stop=(vt == VT - 1))
                nc.vector.tensor_copy(xc_sb[:, jt, :], ps[:])

            xct_sb = bigsc.tile([128, WCT, J], F32R, tag="bigsc")
            for wct in range(WCT):
                for jc in range(2):
                    ps = pd.tile([128, 384], F32, tag="pd")
                    for vt in range(VT):
                        nc.tensor.matmul(
                            ps[:], afc_sb[:, vt, wct * 128:(wct + 1) * 128],
                            xj_sb[:, vt, jc * 384:(jc + 1) * 384],
                            start=(vt == 0), stop=(vt == VT - 1))
                    nc.vector.tensor_copy(xct_sb[:, wct, jc * 384:(jc + 1) * 384], ps[:])

            sxg_sb = small.tile([128, JT, S], F32, tag="sxg")
            for jt in range(JT):
                ps = pd.tile([128, S], F32, tag="pd")
                for wct in range(WCT):
                    nc.tensor.matmul(ps[:], xct_sb[:, wct, jt * 128:(jt + 1) * 128],
                                     acs_r[:, wct, :],
                                     start=(wct == 0), stop=(wct == WCT - 1))
                nc.vector.tensor_copy(sxg_sb[:, jt, :], ps[:])

            sxgt_sb = small.tile([S, J], F32, tag="sxgt")
            for jc in range(2):
                ps = pd.tile([S, 384], F32, tag="pd")
                for wct in range(WCT):
                    nc.tensor.matmul(ps[:], acs_r[:, wct, :],
                                     xct_sb[:, wct, jc * 384:(jc + 1) * 384],
                                     start=(wct == 0), stop=(wct == WCT - 1))
                nc.vector.tensor_copy(sxgt_sb[:, jc * 384:(jc + 1) * 384], ps[:])

            # ---- allgather sxg ----
            for jt in range(JT):
                nc.sync.dma_start(cc_in.ap()[jt * 128:(jt + 1) * 128, :], sxg_sb[:, jt, :])
            nc.gpsimd.collective_compute(
                "AllGather", mybir.AluOpType.bypass,
                replica_groups=[list(range(NCORES))],
                ins=[cc_in.ap()], outs=[cc_out.ap()],
            )

            # ---- as_mat = P1(SXG).T @ P2(SXG), enumerated in SXG-row order ----
            as_ps = pd.tile([S, S], F32, tag="pd")
            for t in range(AT):
                a1t = agp.tile([128, S], F32, tag="a1t")
                nc.sync.dma_start(a1t[:], cc_out.ap()[t * 128:(t + 1) * 128, :])
                a2t = agp.tile([128, S], F32, tag="a2t")
                nc.gpsimd.indirect_dma_start(
                    out=a2t[:], out_offset=None,
                    in_=cc_out.ap(),
                    in_offset=bass.IndirectOffsetOnAxis(ap=a2i_sb[:, t:t + 1], axis=0),
                )
                nc.tensor.matmul(as_ps[:], a1t[:], a2t[:],
                                 start=(t == 0), stop=(t == AT - 1))

            # relu(as_mat - 0.5), then transpose
            asm = small.tile([S, S], F32, tag="asm")
            nc.scalar.activation(asm[:], as_ps[:],
                                 mybir.ActivationFunctionType.Relu, bias=neg_half[:S, :])
            asmt_ps = ptr.tile([S, S], F32, tag="ptr")
            nc.tensor.transpose(asmt_ps[:], asm[:], ident[:S, :S])
            asmt = small.tile([S, S], F32, tag="asmt")
            nc.vector.tensor_copy(asmt[:], asmt_ps[:])

            # asym_adj + softmax (rows = partitions)
            sups = []
            for mi, m in enumerate((asm, asmt)):
                rs = small.tile([S, 1], F32, tag=f"rs{mi}")
                nc.vector.tensor_reduce(rs[:], m[:], mybir.AxisListType.X,
                                        mybir.AluOpType.add)
                nc.vector.tensor_scalar_max(rs[:], rs[:], 1e-30)
                rinv = small.tile([S, 1], F32, tag=f"rinv{mi}")
                nc.vector.reciprocal(rinv[:], rs[:])
                mn = small.tile([S, S], F32, tag=f"mn{mi}")
                nc.vector.tensor_scalar_mul(mn[:], m[:], rinv[:])
                mx = small.tile([S, 1], F32, tag=f"mx{mi}")
                nc.vector.tensor_reduce(mx[:], mn[:], mybir.AxisListType.X,
                                        mybir.AluOpType.max)
                nmx = small.tile([S, 1], F32, tag=f"nmx{mi}")
                nc.vector.tensor_scalar_mul(nmx[:], mx[:], -1.0)
                e = small.tile([S, S], F32, tag=f"e{mi}")
                nc.scalar.activation(e[:], mn[:],
                                     mybir.ActivationFunctionType.Exp, bias=nmx[:])
                se = small.tile([S, 1], F32, tag=f"se{mi}")
                nc.vector.tensor_reduce(se[:], e[:], mybir.AxisListType.X,
                                        mybir.AluOpType.add)
                sinv = small.tile([S, 1], F32, tag=f"sinv{mi}")
                nc.vector.reciprocal(sinv[:], se[:])
                sup = small.tile([S, S], F32, tag=f"sup{mi}")
                nc.vector.tensor_scalar_mul(sup[:], e[:], sinv[:])
                sups.append(sup)
                if debug:
                    nc.sync.dma_start(dbg_asm_d.ap()[:, mi * S:(mi + 1) * S], sup[:])

            # ---- super diffusion + conv -> hs_pre ----
            ys_tiles = {}
            for k in range(2):
                for jt in range(JT):
                    ps = pd.tile([128, S], F32, tag="pd")
                    nc.tensor.matmul(ps[:], sxgt_sb[:, jt * 128:(jt + 1) * 128],
                                     sups[k][:], start=True, stop=True)
                    yt = ys.tile([128, S], F32, tag="ys")
                    nc.vector.tensor_copy(yt[:], ps[:])
                    ys_tiles[(k, jt)] = yt

            hs_pre = small.tile([128, JT, S], F32, tag="hs_pre")
            for jt in range(JT):
                ps = pcf.tile([128, S], F32, tag="pcf")
                parts = [sxg_sb[:, jt, :], ys_tiles[(0, jt)][:], ys_tiles[(1, jt)][:]]
                for p, rhs_full in enumerate(parts):
                    for g in range(4):
                        nc.tensor.matmul(
                            ps[32 * g:32 * (g + 1), :],
                            ws32[32 * g:32 * (g + 1), 32 * p:32 * (p + 1)],
                            rhs_full[32 * g:32 * (g + 1), :],
                            start=(p == 0), stop=(p == 2),
                            tile_position=(32 * g, 32 * g))
                nc.vector.tensor_scalar_add(hs_pre[:, jt, :], ps[:], b128[:])

            # ---- coarse diffusion + conv -> hc_pre ----
            yc_tiles = {}
            for k in range(2):
                for jt in range(JT):
                    ps = pd.tile([128, VC], F32, tag="pd")
                    for wct in range(WCT):
                        nc.tensor.matmul(ps[:], xct_sb[:, wct, jt * 128:(jt + 1) * 128],
                                         supc[:, k, wct, :],
                                         start=(wct == 0), stop=(wct == WCT - 1))
                    yt = ys.tile([128, VC], dt.bfloat16, tag="yc")
                    nc.vector.tensor_copy(yt[:], ps[:])
                    yc_tiles[(k, jt)] = yt

            hc_pre = bigsc.tile([128, JT, VC], F32, tag="bigsc")
            for jt in range(JT):
                ps = pcf.tile([128, VC], F32, tag="pcf")
                parts = [xc_sb[:, jt, :], yc_tiles[(0, jt)][:], yc_tiles[(1, jt)][:]]
                for p, rhs_full in enumerate(parts):
                    wsel = ws32 if p == 0 else wsbf
                    for g in range(4):
                        nc.tensor.matmul(
                            ps[32 * g:32 * (g + 1), :],
                            wsel[32 * g:32 * (g + 1), 32 * p:32 * (p + 1)],
                            rhs_full[32 * g:32 * (g + 1), :],
                            start=(p == 0), stop=(p == 2),
                            tile_position=(32 * g, 32 * g))
                nc.vector.tensor_scalar_add(hc_pre[:, jt, :], ps[:], b128[:])

            # ---- step 3: hc1 = hc_pre + N1*relu(acs @ hs_pre) ----
            hspt = small.tile([S, J], F32, tag="hspt")
            for jt in range(JT):
                tp = ptr.tile([S, 128], F32, tag="ptr")
                nc.tensor.transpose(tp[:], hs_pre[:, jt, :], ident[:])
                nc.vector.tensor_copy(hspt[:, jt * 128:(jt + 1) * 128], tp[:])

            hc1 = bigsc.tile([128, JT, VC], F32, tag="bigsc")
            hc1t = small.tile([128, WCT, J], F32R, tag="hc1t")
            for jt in range(JT):
                ps = pcf.tile([128, VC], F32, tag="pcf")
                nc.tensor.matmul(ps[:], hspt[:, jt * 128:(jt + 1) * 128],
                                 acst32[:], start=True, stop=True)
                rl = works.tile([128, VC], F32, tag="rl3")
                nc.scalar.activation(rl[:], ps[:],
                                     mybir.ActivationFunctionType.Relu, scale=sc_n1[:])
                nc.vector.tensor_add(hc1[:, jt, :], hc_pre[:, jt, :], rl[:])
                for wct in range(WCT):
                    tp = ptr.tile([128, 128], F32, tag="ptr")
                    nc.tensor.transpose(tp[:], hc1[:, jt, wct * 128:(wct + 1) * 128],
                                        ident[:])
                    nc.vector.tensor_copy(
                        hc1t[:, wct, jt * 128:(jt + 1) * 128], tp[:])

            if debug:
                for jt in range(JT):
                    nc.sync.dma_start(dbg_hc1_d.ap()[jt * 128:(jt + 1) * 128, :], hc1[:, jt, :])
                    nc.sync.dma_start(dbg_hspre_d.ap()[jt * 128:(jt + 1) * 128, :], hs_pre[:, jt, :])
                    nc.sync.dma_start(dbg_sxg_d.ap()[jt * 128:(jt + 1) * 128, :], sxg_sb[:, jt, :])

            # ---- fine stage: diffusion + conv + step4 fusion + step5 accum ----
            acc5 = small.tile([128, JT, VC], F32, tag="acc5")
            nc.gpsimd.memset(acc5[:], 0.0)
            for w in range(NW):
                ablk = {}
                for k, ad in enumerate((a0_d, a1_d)):
                    for vt in range(VT):
                        at = astr.tile([128, WCH], F32R, tag="ablk")
                        nc.sync.dma_start(
                            at[:], r(ad.ap()[vt * 128:(vt + 1) * 128,
                                             w * WCH:(w + 1) * WCH]))
                        ablk[(k, vt)] = at
                afct_blk = {}
                for wct in range(WCT):
                    at = afctstr.tile([128, WCH], F32R, tag="afctblk")
                    nc.sync.dma_start(
                        at[:], r(afct_d.ap()[wct * 128:(wct + 1) * 128,
                                             w * WCH:(w + 1) * WCH]))
                    afct_blk[wct] = at

                for jt in range(JT):
                    xt_t = xtstr.tile([128, WCH], F32, tag="xtblk")
                    nc.sync.dma_start(
                        xt_t[:], xt_d.ap()[jt * 128:(jt + 1) * 128,
                                           w * WCH:(w + 1) * WCH])

                    # stage 1: two diffusions
                    ycopies = []
                    for k in range(2):
                        psd = pd.tile([128, WCH], F32, tag="pd")
                        for vt in range(VT):
                            nc.tensor.matmul(
                                psd[:], xj_sb[:, vt, jt * 128:(jt + 1) * 128],
                                ablk[(k, vt)][:],
                                start=(vt == 0), stop=(vt == VT - 1))
                        yc = workf.tile([128, WCH], dt.bfloat16, tag="ycopy")
                        nc.vector.tensor_copy(yc[:], psd[:])
                        ycopies.append(yc)

                    # stage 2 conv + stage 4 afc-term (separate psum slices)
                    pscf = pcf.tile([128, 2 * WCH], F32, tag="pcf")
                    psc = pscf[:, :WCH]
                    psf = pscf[:, WCH:]
                    parts = [xt_t[:], ycopies[0][:], ycopies[1][:]]
                    for p, rhs_full in enumerate(parts):
                        wsel = ws32 if p == 0 else wsbf
                        for g in range(4):
                            nc.tensor.matmul(
                                psc[32 * g:32 * (g + 1), :],
                                wsel[32 * g:32 * (g + 1), 32 * p:32 * (p + 1)],
                                rhs_full[32 * g:32 * (g + 1), :],
                                start=(p == 0), stop=(p == 2),
                                tile_position=(32 * g, 32 * g))
                    for wct in range(WCT):
                        nc.tensor.matmul(psf[:],
                                         hc1t[:, wct, jt * 128:(jt + 1) * 128],
                                         afct_blk[wct][:],
                                         start=(wct == 0), stop=(wct == WCT - 1))

                    rl = workf.tile([128, WCH], F32, tag="rl4")
                    nc.scalar.activation(rl[:], psf[:],
                                         mybir.ActivationFunctionType.Relu, scale=sc_02[:])
                    hft = workf.tile([128, WCH], F32, tag="hfres")
                    nc.vector.tensor_add(hft[:], psc[:], rl[:])
                    nc.vector.tensor_scalar_add(hft[:], hft[:], b128[:])
                    nc.sync.dma_start(
                        hf_d.ap()[jt * 128:(jt + 1) * 128, w * WCH:(w + 1) * WCH],
                        hft[:])

                    # step 5 accumulation: transpose hf tile, multiply by afc,
                    # accumulate into SBUF via DVE (PSUM start= clears are
                    # bank-granular, so a persistent multi-slice PSUM
                    # accumulator is unsafe)
                    ps5t = p5p.tile([128, VC], F32, tag="p5")
                    for sub in range(WCH // 128):
                        wt = w * (WCH // 128) + sub
                        tp = ptr.tile([128, 128], F32, tag="ptr")
                        nc.tensor.transpose(
                            tp[:], hft[:, sub * 128:(sub + 1) * 128], ident[:])
                        hftr = workf.tile([128, 128], F32R, tag="hftr")
                        nc.vector.tensor_copy(hftr[:], tp[:])
                        nc.tensor.matmul(ps5t[:], hftr[:],
                                         afc_sb[:, wt, :],
                                         start=(sub == 0), stop=(sub == 1))
                    nc.vector.tensor_add(acc5[:, jt, :], acc5[:, jt, :], ps5t[:])

            # ---- step 5 finish: hc2 = hc1 + N3*relu(ps5); output hc ----
            hc2 = bigsc.tile([128, JT, VC], F32, tag="bigsc")
            for jt in range(JT):
                rl = works.tile([128, VC], F32, tag="rl5")
                nc.scalar.activation(rl[:], acc5[:, jt, :],
                                     mybir.ActivationFunctionType.Relu, scale=sc_02[:])
                nc.vector.tensor_add(hc2[:, jt, :], hc1[:, jt, :], rl[:])
                if debug:
                    nc.sync.dma_start(dbg_t5_d.ap()[jt * 128:(jt + 1) * 128, :], rl[:])
                nc.sync.dma_start(hc_d.ap()[jt * 128:(jt + 1) * 128, :], hc2[:, jt, :])

            # ---- step 6: hs_out = hs_pre + N4*relu(acs.T @ hc2) ----
            for jt in range(JT):
                ps = pcf.tile([128, S], F32, tag="pcf")
                for wct in range(WCT):
                    tp = ptr.tile([128, 128], F32, tag="ptr")
                    nc.tensor.transpose(tp[:], hc2[:, jt, wct * 128:(wct + 1) * 128],
                                        ident[:])
                    h2t = works.tile([128, 128], F32, tag="h2t")
                    nc.vector.tensor_copy(h2t[:], tp[:])
                    nc.tensor.matmul(ps[:], h2t[:], acs32[:, wct, :],
                                     start=(wct == 0), stop=(wct == WCT - 1))
                rl = works.tile([128, S], F32, tag="rl6")
                nc.scalar.activation(rl[:], ps[:],
                                     mybir.ActivationFunctionType.Relu, scale=sc_02[:])
                hso = works.tile([128, S], F32, tag="hso")
                nc.vector.tensor_add(hso[:], hs_pre[:, jt, :], rl[:])
                nc.sync.dma_start(hs_d.ap()[jt * 128:(jt + 1) * 128, :], hso[:])

    nc.compile()
    return nc


def _a2_indices() -> np.ndarray:
    """Row gather indices: position enumerated by left row Lr (SXG row order);
    right row = lmajor-unflatten of the cmajor position index."""
    Lr = np.arange(NCORES * J)
    n = Lr // (L * C)
    l = (Lr // C) % L
    c = Lr % C
    i = c * (N * L) + n * L + l          # cmajor position of this left row
    l2 = i // (N * C)
    n2 = (i // C) % N
    c2 = i % C
    Rr = n2 * (L * C) + l2 * C + c2
    return Rr.astype(np.int32).reshape(AT, 128).T.copy()  # [128, AT]


def kernel(x, support, support_c, acs, afc, W, b):
    x = np.asarray(x, np.float32)
    support = np.asarray(support, np.float32)
    support_c = np.asarray(support_c, np.float32)
    acs = np.asarray(acs, np.float32)
    afc = np.asarray(afc, np.float32)
    W = np.asarray(W, np.float32)
    b = np.asarray(b, np.float32)

    if "nc" not in _CACHE:
        _CACHE["nc"] = _build()
    nc = _CACHE["nc"]

    wstack = np.zeros((128, 96), np.float32)
    for g in range(4):
        for p in range(3):
            wstack[32 * g:32 * (g + 1), 32 * p:32 * (p + 1)] = W[:, 32 * p:32 * (p + 1)].T
    b128 = b[np.arange(128) % 32].reshape(128, 1).astype(np.float32)
    a2idx = _a2_indices()

    shared = {
        "a0": np.ascontiguousarray(support[0]),
        "a1": np.ascontiguousarray(support[1]),
        "c0": np.ascontiguousarray(support_c[0]),
        "c1": np.ascontiguousarray(support_c[1]),
        "afc": afc,
        "afct": np.ascontiguousarray(afc.T),
        "acs": acs,
        "acst": np.ascontiguousarray(acs.T),
        "wstack": wstack,
        "b128": b128,
        "a2idx": a2idx,
    }
    in_maps = []
    for i in range(NCORES):
        xs = x[NLOC * i:NLOC * (i + 1)]
        in_maps.append(dict(
            shared,
            xj=np.ascontiguousarray(xs.transpose(2, 0, 3, 1).reshape(V, J)),
            xt=np.ascontiguousarray(xs.transpose(0, 3, 1, 2).reshape(J, V)),
        ))

    trace = bool(os.environ.get("HGCN_TRACE"))
    if trace:
        try:
            import ntff_shim  # noqa: F401
        except Exception:
            pass
    res = run_bass_kernel_spmd(nc, in_maps, list(range(NCORES)), trace=trace)
    _CACHE["last_result"] = res

    hf = np.empty((N, CO, V, L), np.float32)
    hc = np.empty((N, CO, VC, L), np.float32)
    hs = np.empty((N, CO, S, L), np.float32)
    for i in range(NCORES):
        sl = slice(NLOC * i, NLOC * (i + 1))
        hf[sl] = res.results[i]["hf"].reshape(NLOC, L, CO, V).transpose(0, 2, 3, 1)
        hc[sl] = res.results[i]["hc"].reshape(NLOC, L, CO, VC).transpose(0, 2, 3, 1)
        hs[sl] = res.results[i]["hs"].reshape(NLOC, L, CO, S).transpose(0, 2, 3, 1)
    return hf, hc, hs


# revision 8
# speedup vs baseline: 1.0149x; 1.0149x over previous
"""HGCN Bass/Trainium2 kernel — 8-core SPMD, data-parallel over batch N.

Strategy:
  - Shard batch N=16 over 8 cores (2 per core); replicate adjacencies + weights.
  - Per-core layout: j = (n, l, c) flattened (row = n*384 + l*32 + c), J = 768.
    Fine diffusion runs as out[j, w] = sum_v xj[v, j] * A[v, w] with xj resident
    in SBUF as the stationary operand (fp32r) and support streamed from HBM.
  - 1x1 conv (contraction over channels c, which live inside the partition dim)
    uses 4-group diagonal tile_position matmuls with a stacked weight tile.
  - The super-adjacency as_mat couples all batches (mismatched torch flatten
    orders), so sxg is AllGathered (196KB/rank) and as_mat computed redundantly
    per core; the right operand's row permutation is applied with an
    indirect-DMA gather using host-precomputed indices.
  - Hierarchical relu fusions: afc/acs contractions via PE with on-the-fly PE
    transposes; step-5 (afc.T @ hf) accumulates into a persistent 3-bank PSUM
    tensor while hf tiles stream out.
"""
import os
import sys

sys.path.insert(0, "/opt/trn_rl_repo")

import numpy as np
from contextlib import ExitStack

import concourse.bass as bass
import concourse.tile as tile
from concourse import bacc, mybir
from concourse.bass_utils import run_bass_kernel_spmd
from concourse.masks import make_identity

dt = mybir.dt
F32 = dt.float32
F32R = dt.float32r

# dims
N, C, V, VC, S, L, CO = 16, 32, 2048, 256, 64, 12, 32
N1, N2, N3, N4 = 0.8, 0.2, 0.2, 0.2
NCORES = 8
NLOC = N // NCORES            # 2 batches per core
J = NLOC * L * C              # 768 local (n,l,c) columns
JT = J // 128                 # 6 j-tiles
VT = V // 128                 # 16 v-tiles
WCH = 256                     # fine w chunk (free dim)
NW = V // WCH                 # 8 w chunks
WCT = VC // 128               # 2 coarse-node tiles
AT = (NCORES * J) // 128      # 48 as_mat contraction tiles

_CACHE = {}


def _build():
    from concourse.compiler_utils import get_compiler_flags, set_compiler_flags
    set_compiler_flags([f.replace("--enable-ldw-opt=false", "--enable-ldw-opt=true")
                        for f in get_compiler_flags()])
    nc = bacc.Bacc("TRN2", target_bir_lowering=False, debug=False,
                   num_devices=NCORES)

    # ---- I/O ----
    xj_d = nc.declare_dram_parameter("xj", [V, J], F32, isOutput=False)
    xt_d = nc.declare_dram_parameter("xt", [J, V], F32, isOutput=False)
    a0_d = nc.declare_dram_parameter("a0", [V, V], F32, isOutput=False)
    a1_d = nc.declare_dram_parameter("a1", [V, V], F32, isOutput=False)
    c0_d = nc.declare_dram_parameter("c0", [VC, VC], F32, isOutput=False)
    c1_d = nc.declare_dram_parameter("c1", [VC, VC], F32, isOutput=False)
    afc_d = nc.declare_dram_parameter("afc", [V, VC], F32, isOutput=False)
    afct_d = nc.declare_dram_parameter("afct", [VC, V], F32, isOutput=False)
    acs_d = nc.declare_dram_parameter("acs", [VC, S], F32, isOutput=False)
    acst_d = nc.declare_dram_parameter("acst", [S, VC], F32, isOutput=False)
    ws_d = nc.declare_dram_parameter("wstack", [128, 96], F32, isOutput=False)
    b128_d = nc.declare_dram_parameter("b128", [128, 1], F32, isOutput=False)
    a2i_d = nc.declare_dram_parameter("a2idx", [128, AT], dt.int32, isOutput=False)

    debug = bool(os.environ.get("HGCN_DEBUG"))
    hf_d = nc.declare_dram_parameter("hf", [J, V], F32, isOutput=True)
    hc_d = nc.declare_dram_parameter("hc", [J, VC], F32, isOutput=True)
    hs_d = nc.declare_dram_parameter("hs", [J, S], F32, isOutput=True)

    if debug:
        dbg_asm_d = nc.declare_dram_parameter("dbg_asm", [S, 2 * S], F32, isOutput=True)
        dbg_hc1_d = nc.declare_dram_parameter("dbg_hc1", [J, VC], F32, isOutput=True)
        dbg_hspre_d = nc.declare_dram_parameter("dbg_hspre", [J, S], F32, isOutput=True)
        dbg_t5_d = nc.declare_dram_parameter("dbg_t5", [J, VC], F32, isOutput=True)
        dbg_sxg_d = nc.declare_dram_parameter("dbg_sxg", [J, S], F32, isOutput=True)
    cc_in = nc.dram_tensor("cc_in", [J, S], F32)
    cc_out = nc.dram_tensor("cc_out", [NCORES * J, S], F32, addr_space="Shared")

    def r(ap):
        return ap.bitcast(F32R)

    with tile.TileContext(nc) as tc:
        with ExitStack() as ctx:
            # ---- pools ----
            const = ctx.enter_context(tc.tile_pool(name="const", bufs=1))
            bigsc = ctx.enter_context(tc.tile_pool(name="bigsc", bufs=3))
            small = ctx.enter_context(tc.tile_pool(name="small", bufs=1))
            astr = ctx.enter_context(tc.tile_pool(name="astr", bufs=34))
            xtstr = ctx.enter_context(tc.tile_pool(name="xtstr", bufs=6))
            afctstr = ctx.enter_context(tc.tile_pool(name="afctstr", bufs=4))
            workf = ctx.enter_context(tc.tile_pool(name="workf", bufs=4))
            works = ctx.enter_context(tc.tile_pool(name="works", bufs=2))
            ys = ctx.enter_context(tc.tile_pool(name="ys", bufs=13))
            agp = ctx.enter_context(tc.tile_pool(name="agp", bufs=8))

            pd = ctx.enter_context(tc.tile_pool(name="pd", bufs=2, space="PSUM"))
            pcf = ctx.enter_context(tc.tile_pool(name="pcf", bufs=2, space="PSUM"))
            ptr = ctx.enter_context(tc.tile_pool(name="ptr", bufs=2, space="PSUM"))
            p5p = ctx.enter_context(tc.tile_pool(name="p5p", bufs=2, space="PSUM"))

            # ---- constants / resident loads ----
            ident = const.tile([128, 128], F32, tag="ident")
            make_identity(nc, ident[:])

            xj_sb = const.tile([128, VT, J], F32R, tag="xj")
            for vt in range(VT):
                nc.sync.dma_start(xj_sb[:, vt, :], r(xj_d.ap()[vt * 128:(vt + 1) * 128, :]))

            afc_sb = const.tile([128, VT, VC], F32R, tag="afc")
            for vt in range(VT):
                nc.sync.dma_start(afc_sb[:, vt, :], r(afc_d.ap()[vt * 128:(vt + 1) * 128, :]))

            acs_r = const.tile([128, WCT, S], F32R, tag="acs_r")
            acs32 = const.tile([128, WCT, S], F32, tag="acs32")
            for wct in range(WCT):
                nc.sync.dma_start(acs_r[:, wct, :], r(acs_d.ap()[wct * 128:(wct + 1) * 128, :]))
                nc.sync.dma_start(acs32[:, wct, :], acs_d.ap()[wct * 128:(wct + 1) * 128, :])
            acst32 = const.tile([S, VC], F32, tag="acst32")
            nc.sync.dma_start(acst32[:], acst_d.ap())

            supc = const.tile([128, 2, WCT, VC], F32R, tag="supc")
            for k, cd in enumerate((c0_d, c1_d)):
                for wct in range(WCT):
                    nc.sync.dma_start(supc[:, k, wct, :], r(cd.ap()[wct * 128:(wct + 1) * 128, :]))

            wsbf = const.tile([128, 96], dt.bfloat16, tag="wsbf")
            ws_f32tmp = const.tile([128, 96], F32, tag="ws_f32tmp")
            nc.sync.dma_start(ws_f32tmp[:], ws_d.ap())
            nc.vector.tensor_copy(wsbf[:], ws_f32tmp[:])
            ws32 = const.tile([128, 96], F32, tag="ws32")
            nc.sync.dma_start(ws32[:], ws_d.ap())
            b128 = const.tile([128, 1], F32, tag="b128")
            nc.sync.dma_start(b128[:], b128_d.ap())
            a2i_sb = const.tile([128, AT], dt.int32, tag="a2i")
            nc.sync.dma_start(a2i_sb[:], a2i_d.ap())
            neg_half = const.tile([128, 1], F32, tag="neg_half")
            nc.gpsimd.memset(neg_half[:], -0.5)
            sc_n1 = const.tile([128, 1], F32, tag="sc_n1")
            nc.gpsimd.memset(sc_n1[:], N1)
            sc_02 = const.tile([128, 1], F32, tag="sc_02")
            nc.gpsimd.memset(sc_02[:], N2)

            # ---- phase A: xc, xcT, sxg, sxgT ----
            xc_sb = bigsc.tile([128, JT, VC], F32, tag="bigsc")
            for jt in range(JT):
                ps = pd.tile([128, VC], F32, tag="pd")
                for vt in range(VT):
                    nc.tensor.matmul(ps[:], xj_sb[:, vt, jt * 128:(jt + 1) * 128],
                                     afc_sb[:, vt, :],
                                     start=(vt == 0), stop=(vt == VT - 1))
                nc.vector.tensor_copy(xc_sb[:, jt, :], ps[:])

            xct_sb = bigsc.tile([128, WCT, J], F32R, tag="bigsc")
            for wct in range(WCT):
                for jc in range(2):
                    ps = pd.tile([128, 384], F32, tag="pd")
                    for vt in range(VT):
                        nc.tensor.matmul(
                            ps[:], afc_sb[:, vt, wct * 128:(wct + 1) * 128],
                            xj_sb[:, vt, jc * 384:(jc + 1) * 384],
                            start=(vt == 0), stop=(vt == VT - 1))
                    nc.vector.tensor_copy(xct_sb[:, wct, jc * 384:(jc + 1) * 384], ps[:])

            sxg_sb = small.tile([128, JT, S], F32, tag="sxg")
            for jt in range(JT):
                ps = pd.tile([128, S], F32, tag="pd")
                for wct in range(WCT):
                    nc.tensor.matmul(ps[:], xct_sb[:, wct, jt * 128:(jt + 1) * 128],
                                     acs_r[:, wct, :],
                                     start=(wct == 0), stop=(wct == WCT - 1))
                nc.vector.tensor_copy(sxg_sb[:, jt, :], ps[:])

            sxgt_sb = small.tile([S, J], F32, tag="sxgt")
            for jc in range(2):
                ps = pd.tile([S, 384], F32, tag="pd")
                for wct in range(WCT):
                    nc.tensor.matmul(ps[:], acs_r[:, wct, :],
                                     xct_sb[:, wct, jc * 384:(jc + 1) * 384],
                                     start=(wct == 0), stop=(wct == WCT - 1))
                nc.vector.tensor_copy(sxgt_sb[:, jc * 384:(jc + 1) * 384], ps[:])

            # ---- allgather sxg ----
            for jt in range(JT):
                nc.sync.dma_start(cc_in.ap()[jt * 128:(jt + 1) * 128, :], sxg_sb[:, jt, :])
            nc.gpsimd.collective_compute(
                "AllGather", mybir.AluOpType.bypass,
                replica_groups=[list(range(NCORES))],
                ins=[cc_in.ap()], outs=[cc_out.ap()],
            )

            # ---- as_mat = P1(SXG).T @ P2(SXG), enumerated in SXG-row order ----
            as_ps = pd.tile([S, S], F32, tag="pd")
            for t in range(AT):
                a1t = agp.tile([128, S], F32, tag="a1t")
                nc.sync.dma_start(a1t[:], cc_out.ap()[t * 128:(t + 1) * 128, :])
                a2t = agp.tile([128, S], F32, tag="a2t")
                nc.gpsimd.indirect_dma_start(
                    out=a2t[:], out_offset=None,
                    in_=cc_out.ap(),
                    in_offset=bass.IndirectOffsetOnAxis(ap=a2i_sb[:, t:t + 1], axis=0),
                )
                nc.tensor.matmul(as_ps[:], a1t[:], a2t[:],
                                 start=(t == 0), stop=(t == AT - 1))

            # relu(as_mat - 0.5), then transpose
            asm = small.tile([S, S], F32, tag="asm")
            nc.scalar.activation(asm[:], as_ps[:],
                                 mybir.ActivationFunctionType.Relu, bias=neg_half[:S, :])
            asmt_ps = ptr.tile([S, S], F32, tag="ptr")
            nc.tensor.transpose(asmt_ps[:], asm[:], ident[:S, :S])
            asmt = small.tile([S, S], F32, tag="asmt")
            nc.vector.tensor_copy(asmt[:], asmt_ps[:])

            # asym_adj + softmax (rows = partitions)
            sups = []
            for mi, m in enumerate((asm, asmt)):
                rs = small.tile([S, 1], F32, tag=f"rs{mi}")
                nc.vector.tensor_reduce(rs[:], m[:], mybir.AxisListType.X,
                                        mybir.AluOpType.add)
                nc.vector.tensor_scalar_max(rs[:], rs[:], 1e-30)
                rinv = small.tile([S, 1], F32, tag=f"rinv{mi}")
                nc.vector.reciprocal(rinv[:], rs[:])
                mn = small.tile([S, S], F32, tag=f"mn{mi}")
                nc.vector.tensor_scalar_mul(mn[:], m[:], rinv[:])
                mx = small.tile([S, 1], F32, tag=f"mx{mi}")
                nc.vector.tensor_reduce(mx[:], mn[:], mybir.AxisListType.X,
                                        mybir.AluOpType.max)
                nmx = small.tile([S, 1], F32, tag=f"nmx{mi}")
                nc.vector.tensor_scalar_mul(nmx[:], mx[:], -1.0)
                e = small.tile([S, S], F32, tag=f"e{mi}")
                nc.scalar.activation(e[:], mn[:],
                                     mybir.ActivationFunctionType.Exp, bias=nmx[:])
                se = small.tile([S, 1], F32, tag=f"se{mi}")
                nc.vector.tensor_reduce(se[:], e[:], mybir.AxisListType.X,
                                        mybir.AluOpType.add)
                sinv = small.tile([S, 1], F32, tag=f"sinv{mi}")
                nc.vector.reciprocal(sinv[:], se[:])
                sup = small.tile([S, S], F32, tag=f"sup{mi}")
                nc.vector.tensor_scalar_mul(sup[:], e[:], sinv[:])
                sups.append(sup)
                if debug:
                    nc.sync.dma_start(dbg_asm_d.ap()[:, mi * S:(mi + 1) * S], sup[:])

            # ---- super diffusion + conv -> hs_pre ----
            ys_tiles = {}
            for k in range(2):
                for jt in range(JT):
                    ps = pd.tile([128, S], F32, tag="pd")
                    nc.tensor.matmul(ps[:], sxgt_sb[:, jt * 128:(jt + 1) * 128],
                                     sups[k][:], start=True, stop=True)
                    yt = ys.tile([128, S], F32, tag="ys")
                    nc.vector.tensor_copy(yt[:], ps[:])
                    ys_tiles[(k, jt)] = yt

            hs_pre = small.tile([128, JT, S], F32, tag="hs_pre")
            for jt in range(JT):
                ps = pcf.tile([128, S], F32, tag="pcf")
                parts = [sxg_sb[:, jt, :], ys_tiles[(0, jt)][:], ys_tiles[(1, jt)][:]]
                for p, rhs_full in enumerate(parts):
                    for g in range(4):
                        nc.tensor.matmul(
                            ps[32 * g:32 * (g + 1), :],
                            ws32[32 * g:32 * (g + 1), 32 * p:32 * (p + 1)],
                            rhs_full[32 * g:32 * (g + 1), :],
                            start=(p == 0), stop=(p == 2),
                            tile_position=(32 * g, 32 * g))
                nc.vector.tensor_scalar_add(hs_pre[:, jt, :], ps[:], b128[:])

            # ---- coarse diffusion + conv -> hc_pre ----
            yc_tiles = {}
            for k in range(2):
                for jt in range(JT):
                    ps = pd.tile([128, VC], F32, tag="pd")
                    for wct in range(WCT):
                        nc.tensor.matmul(ps[:], xct_sb[:, wct, jt * 128:(jt + 1) * 128],
                                         supc[:, k, wct, :],
                                         start=(wct == 0), stop=(wct == WCT - 1))
                    yt = ys.tile([128, VC], dt.bfloat16, tag="yc")
                    nc.vector.tensor_copy(yt[:], ps[:])
                    yc_tiles[(k, jt)] = yt

            hc_pre = bigsc.tile([128, JT, VC], F32, tag="bigsc")
            for jt in range(JT):
                ps = pcf.tile([128, VC], F32, tag="pcf")
                parts = [xc_sb[:, jt, :], yc_tiles[(0, jt)][:], yc_tiles[(1, jt)][:]]
                for p, rhs_full in enumerate(parts):
                    wsel = ws32 if p == 0 else wsbf
                    for g in range(4):
                        nc.tensor.matmul(
                            ps[32 * g:32 * (g + 1), :],
                            wsel[32 * g:32 * (g + 1), 32 * p:32 * (p + 1)],
                            rhs_full[32 * g:32 * (g + 1), :],
                            start=(p == 0), stop=(p == 2),
                            tile_position=(32 * g, 32 * g))
                nc.vector.tensor_scalar_add(hc_pre[:, jt, :], ps[:], b128[:])

            # ---- step 3: hc1 = hc_pre + N1*relu(acs @ hs_pre) ----
            hspt = small.tile([S, J], F32, tag="hspt")
            for jt in range(JT):
                tp = ptr.tile([S, 128], F32, tag="ptr")
                nc.tensor.transpose(tp[:], hs_pre[:, jt, :], ident[:])
                nc.vector.tensor_copy(hspt[:, jt * 128:(jt + 1) * 128], tp[:])

            hc1 = bigsc.tile([128, JT, VC], F32, tag="bigsc")
            hc1t = small.tile([128, WCT, J], F32R, tag="hc1t")
            for jt in range(JT):
                ps = pcf.tile([128, VC], F32, tag="pcf")
                nc.tensor.matmul(ps[:], hspt[:, jt * 128:(jt + 1) * 128],
                                 acst32[:], start=True, stop=True)
                rl = works.tile([128, VC], F32, tag="rl3")
                nc.scalar.activation(rl[:], ps[:],
                                     mybir.ActivationFunctionType.Relu, scale=sc_n1[:])
                nc.vector.tensor_add(hc1[:, jt, :], hc_pre[:, jt, :], rl[:])
                for wct in range(WCT):
                    tp = ptr.tile([128, 128], F32, tag="ptr")
                    nc.tensor.transpose(tp[:], hc1[:, jt, wct * 128:(wct + 1) * 128],
                                        ident[:])
                    nc.vector.tensor_copy(
                        hc1t[:, wct, jt * 128:(jt + 1) * 128], tp[:])

            if debug:
                for jt in range(JT):
                    nc.sync.dma_start(dbg_hc1_d.ap()[jt * 128:(jt + 1) * 128, :], hc1[:, jt, :])
                    nc.sync.dma_start(dbg_hspre_d.ap()[jt * 128:(jt + 1) * 128, :], hs_pre[:, jt, :])
                    nc.sync.dma_start(dbg_sxg_d.ap()[jt * 128:(jt + 1) * 128, :], sxg_sb[:, jt, :])

            # ---- fine stage: diffusion + conv + step4 fusion + step5 accum ----
            acc5 = small.tile([128, JT, VC], F32, tag="acc5")
            nc.gpsimd.memset(acc5[:], 0.0)
            for w in range(NW):
                ablk = {}
                for k, ad in enumerate((a0_d, a1_d)):
                    for vt in range(VT):
                        at = astr.tile([128, WCH], F32R, tag="ablk")
                        nc.sync.dma_start(
                            at[:], r(ad.ap()[vt * 128:(vt + 1) * 128,
                                             w * WCH:(w + 1) * WCH]))
                        ablk[(k, vt)] = at
                afct_blk = {}
                for wct in range(WCT):
                    at = afctstr.tile([128, WCH], F32R, tag="afctblk")
                    nc.sync.dma_start(
                        at[:], r(afct_d.ap()[wct * 128:(wct + 1) * 128,
                                             w * WCH:(w + 1) * WCH]))
                    afct_blk[wct] = at

                for jt in range(JT):
                    xt_t = xtstr.tile([128, WCH], F32, tag="xtblk")
                    nc.sync.dma_start(
                        xt_t[:], xt_d.ap()[jt * 128:(jt + 1) * 128,
                                           w * WCH:(w + 1) * WCH])

                    # stage 1: two diffusions, interleaved so both matmuls
                    # of each v-tile share one weight load (ldw-opt)
                    psd0 = pd.tile([128, WCH], F32, tag="pd")
                    psd1 = pd.tile([128, WCH], F32, tag="pd")
                    for vt in range(VT):
                        lhs = xj_sb[:, vt, jt * 128:(jt + 1) * 128]
                        nc.tensor.matmul(psd0[:], lhs, ablk[(0, vt)][:],
                                         start=(vt == 0), stop=(vt == VT - 1))
                        nc.tensor.matmul(psd1[:], lhs, ablk[(1, vt)][:],
                                         start=(vt == 0), stop=(vt == VT - 1))
                    ycopies = []
                    for psd in (psd0, psd1):
                        yc = workf.tile([128, WCH], dt.bfloat16, tag="ycopy")
                        nc.vector.tensor_copy(yc[:], psd[:])
                        ycopies.append(yc)

                    # stage 2 conv + stage 4 afc-term (separate psum slices)
                    pscf = pcf.tile([128, 2 * WCH], F32, tag="pcf")
                    psc = pscf[:, :WCH]
                    psf = pscf[:, WCH:]
                    parts = [xt_t[:], ycopies[0][:], ycopies[1][:]]
                    for p, rhs_full in enumerate(parts):
                        wsel = ws32 if p == 0 else wsbf
                        for g in range(4):
                            nc.tensor.matmul(
                                psc[32 * g:32 * (g + 1), :],
                                wsel[32 * g:32 * (g + 1), 32 * p:32 * (p + 1)],
                                rhs_full[32 * g:32 * (g + 1), :],
                                start=(p == 0), stop=(p == 2),
                                tile_position=(32 * g, 32 * g))
                    for wct in range(WCT):
                        nc.tensor.matmul(psf[:],
                                         hc1t[:, wct, jt * 128:(jt + 1) * 128],
                                         afct_blk[wct][:],
                                         start=(wct == 0), stop=(wct == WCT - 1))

                    rl = workf.tile([128, WCH], F32, tag="rl4")
                    nc.scalar.activation(rl[:], psf[:],
                                         mybir.ActivationFunctionType.Relu, scale=sc_02[:])
                    hft = workf.tile([128, WCH], F32, tag="hfres")
                    nc.vector.tensor_add(hft[:], psc[:], rl[:])
                    nc.vector.tensor_scalar_add(hft[:], hft[:], b128[:])
                    nc.sync.dma_start(
                        hf_d.ap()[jt * 128:(jt + 1) * 128, w * WCH:(w + 1) * WCH],
                        hft[:])

                    # step 5 accumulation: transpose hf tile, multiply by afc,
                    # accumulate into SBUF via DVE (PSUM start= clears are
                    # bank-granular, so a persistent multi-slice PSUM
                    # accumulator is unsafe)
                    ps5t = p5p.tile([128, VC], F32, tag="p5")
                    for sub in range(WCH // 128):
                        wt = w * (WCH // 128) + sub
                        tp = ptr.tile([128, 128], F32, tag="ptr")
                        nc.tensor.transpose(
                            tp[:], hft[:, sub * 128:(sub + 1) * 128], ident[:])
                        hftr = workf.tile([128, 128], F32R, tag="hftr")
                        nc.vector.tensor_copy(hftr[:], tp[:])
                        nc.tensor.matmul(ps5t[:], hftr[:],
                                         afc_sb[:, wt, :],
                                         start=(sub == 0), stop=(sub == 1))
                    nc.vector.tensor_add(acc5[:, jt, :], acc5[:, jt, :], ps5t[:])

            # ---- step 5 finish: hc2 = hc1 + N3*relu(ps5); output hc ----
            hc2 = bigsc.tile([128, JT, VC], F32, tag="bigsc")
            for jt in range(JT):
                rl = works.tile([128, VC], F32, tag="rl5")
                nc.scalar.activation(rl[:], acc5[:, jt, :],
                                     mybir.ActivationFunctionType.Relu, scale=sc_02[:])
                nc.vector.tensor_add(hc2[:, jt, :], hc1[:, jt, :], rl[:])
                if debug:
                    nc.sync.dma_start(dbg_t5_d.ap()[jt * 128:(jt + 1) * 128, :], rl[:])
                nc.sync.dma_start(hc_d.ap()[jt * 128:(jt + 1) * 128, :], hc2[:, jt, :])

            # ---- step 6: hs_out = hs_pre + N4*relu(acs.T @ hc2) ----
            for jt in range(JT):
                ps = pcf.tile([128, S], F32, tag="pcf")
                for wct in range(WCT):
                    tp = ptr.tile([128, 128], F32, tag="ptr")
                    nc.tensor.transpose(tp[:], hc2[:, jt, wct * 128:(wct + 1) * 128],
                                        ident[:])
                    h2t = works.tile([128, 128], F32, tag="h2t")
                    nc.vector.tensor_copy(h2t[:], tp[:])
                    nc.tensor.matmul(ps[:], h2t[:], acs32[:, wct, :],
                                     start=(wct == 0), stop=(wct == WCT - 1))
                rl = works.tile([128, S], F32, tag="rl6")
                nc.scalar.activation(rl[:], ps[:],
                                     mybir.ActivationFunctionType.Relu, scale=sc_02[:])
                hso = works.tile([128, S], F32, tag="hso")
                nc.vector.tensor_add(hso[:], hs_pre[:, jt, :], rl[:])
                nc.sync.dma_start(hs_d.ap()[jt * 128:(jt + 1) * 128, :], hso[:])

    nc.compile()
    return nc


def _a2_indices() -> np.ndarray:
    """Row gather indices: position enumerated by left row Lr (SXG row order);
    right row = lmajor-unflatten of the cmajor position index."""
    Lr = np.arange(NCORES * J)
    n = Lr // (L * C)
    l = (Lr // C) % L
    c = Lr % C
    i = c * (N * L) + n * L + l          # cmajor position of this left row
    l2 = i // (N * C)
    n2 = (i // C) % N
    c2 = i % C
    Rr = n2 * (L * C) + l2 * C + c2
    return Rr.astype(np.int32).reshape(AT, 128).T.copy()  # [128, AT]


def kernel(x, support, support_c, acs, afc, W, b):
    x = np.asarray(x, np.float32)
    support = np.asarray(support, np.float32)
    support_c = np.asarray(support_c, np.float32)
    acs = np.asarray(acs, np.float32)
    afc = np.asarray(afc, np.float32)
    W = np.asarray(W, np.float32)
    b = np.asarray(b, np.float32)

    if "nc" not in _CACHE:
        _CACHE["nc"] = _build()
    nc = _CACHE["nc"]

    wstack = np.zeros((128, 96), np.float32)
    for g in range(4):
        for p in range(3):
            wstack[32 * g:32 * (g + 1), 32 * p:32 * (p + 1)] = W[:, 32 * p:32 * (p + 1)].T
    b128 = b[np.arange(128) % 32].reshape(128, 1).astype(np.float32)
    a2idx = _a2_indices()

    shared = {
        "a0": np.ascontiguousarray(support[0]),
        "a1": np.ascontiguousarray(support[1]),
        "c0": np.ascontiguousarray(support_c[0]),
        "c1": np.ascontiguousarray(support_c[1]),
        "afc": afc,
        "afct": np.ascontiguousarray(afc.T),
        "acs": acs,
        "acst": np.ascontiguousarray(acs.T),
        "wstack": wstack,
        "b128": b128,
        "a2idx": a2idx,
    }
    in_maps = []
    for i in range(NCORES):
        xs = x[NLOC * i:NLOC * (i + 1)]
        in_maps.append(dict(
            shared,
            xj=np.ascontiguousarray(xs.transpose(2, 0, 3, 1).reshape(V, J)),
            xt=np.ascontiguousarray(xs.transpose(0, 3, 1, 2).reshape(J, V)),
        ))

    trace = bool(os.environ.get("HGCN_TRACE"))
    if trace:
        try:
            import ntff_shim  # noqa: F401
        except Exception:
            pass
    res = run_bass_kernel_spmd(nc, in_maps, list(range(NCORES)), trace=trace)
    _CACHE["last_result"] = res

    hf = np.empty((N, CO, V, L), np.float32)
    hc = np.empty((N, CO, VC, L), np.float32)
    hs = np.empty((N, CO, S, L), np.float32)
    for i in range(NCORES):
        sl = slice(NLOC * i, NLOC * (i + 1))
        hf[sl] = res.results[i]["hf"].reshape(NLOC, L, CO, V).transpose(0, 2, 3, 1)
        hc[sl] = res.results[i]["hc"].reshape(NLOC, L, CO, VC).transpose(0, 2, 3, 1)
        hs[sl] = res.results[i]["hs"].reshape(NLOC, L, CO, S).transpose(0, 2, 3, 1)
    return hf, hc, hs


# revision 11
# speedup vs baseline: 1.0859x; 1.0700x over previous
"""HGCN Bass/Trainium2 kernel — 8-core SPMD, data-parallel over batch N.

Strategy:
  - Shard batch N=16 over 8 cores (2 per core); replicate adjacencies + weights.
  - Per-core layout: j = (n, l, c) flattened (row = n*384 + l*32 + c), J = 768.
    Fine diffusion runs as out[j, w] = sum_v xj[v, j] * A[v, w] with xj resident
    in SBUF as the stationary operand (fp32r) and support streamed from HBM.
  - 1x1 conv (contraction over channels c, which live inside the partition dim)
    uses 4-group diagonal tile_position matmuls with a stacked weight tile.
  - The super-adjacency as_mat couples all batches (mismatched torch flatten
    orders), so sxg is AllGathered (196KB/rank) and as_mat computed redundantly
    per core; the right operand's row permutation is applied with an
    indirect-DMA gather using host-precomputed indices.
  - Hierarchical relu fusions: afc/acs contractions via PE with on-the-fly PE
    transposes; step-5 (afc.T @ hf) accumulates into a persistent 3-bank PSUM
    tensor while hf tiles stream out.
"""
import os
import sys

sys.path.insert(0, "/opt/trn_rl_repo")

import numpy as np
from contextlib import ExitStack

import concourse.bass as bass
import concourse.tile as tile
from concourse import bacc, mybir
from concourse.bass_utils import run_bass_kernel_spmd
from concourse.masks import make_identity

dt = mybir.dt
F32 = dt.float32
F32R = dt.float32r

# dims
N, C, V, VC, S, L, CO = 16, 32, 2048, 256, 64, 12, 32
N1, N2, N3, N4 = 0.8, 0.2, 0.2, 0.2
NCORES = 8
NLOC = N // NCORES            # 2 batches per core
J = NLOC * L * C              # 768 local (n,l,c) columns
JT = J // 128                 # 6 j-tiles
VT = V // 128                 # 16 v-tiles
WCH = 256                     # fine w chunk (free dim)
NW = V // WCH                 # 8 w chunks
WCT = VC // 128               # 2 coarse-node tiles
AT = (NCORES * J) // 128      # 48 as_mat contraction tiles

_CACHE = {}


def _build():
    from concourse.compiler_utils import get_compiler_flags, set_compiler_flags
    set_compiler_flags([f.replace("--enable-ldw-opt=false", "--enable-ldw-opt=true")
                        for f in get_compiler_flags()])
    nc = bacc.Bacc("TRN2", target_bir_lowering=False, debug=False,
                   num_devices=NCORES)

    # ---- I/O ----
    xj_d = nc.declare_dram_parameter("xj", [V, J], F32, isOutput=False)
    xt_d = nc.declare_dram_parameter("xt", [J, V], F32, isOutput=False)
    a0_d = nc.declare_dram_parameter("a0", [V, V], F32, isOutput=False)
    a1_d = nc.declare_dram_parameter("a1", [V, V], F32, isOutput=False)
    c0_d = nc.declare_dram_parameter("c0", [VC, VC], F32, isOutput=False)
    c1_d = nc.declare_dram_parameter("c1", [VC, VC], F32, isOutput=False)
    afc_d = nc.declare_dram_parameter("afc", [V, VC], F32, isOutput=False)
    afct_d = nc.declare_dram_parameter("afct", [VC, V], F32, isOutput=False)
    acs_d = nc.declare_dram_parameter("acs", [VC, S], F32, isOutput=False)
    acst_d = nc.declare_dram_parameter("acst", [S, VC], F32, isOutput=False)
    ws_d = nc.declare_dram_parameter("wstack", [128, 96], F32, isOutput=False)
    b128_d = nc.declare_dram_parameter("b128", [128, 1], F32, isOutput=False)
    a2i_d = nc.declare_dram_parameter("a2idx", [128, AT], dt.int32, isOutput=False)
    afcacs_d = nc.declare_dram_parameter("afcacs", [V, S], F32, isOutput=False)

    debug = bool(os.environ.get("HGCN_DEBUG"))
    hf_d = nc.declare_dram_parameter("hf", [J, V], F32, isOutput=True)
    hc_d = nc.declare_dram_parameter("hc", [J, VC], F32, isOutput=True)
    hs_d = nc.declare_dram_parameter("hs", [J, S], F32, isOutput=True)

    if debug:
        dbg_asm_d = nc.declare_dram_parameter("dbg_asm", [S, 2 * S], F32, isOutput=True)
        dbg_hc1_d = nc.declare_dram_parameter("dbg_hc1", [J, VC], F32, isOutput=True)
        dbg_hspre_d = nc.declare_dram_parameter("dbg_hspre", [J, S], F32, isOutput=True)
        dbg_t5_d = nc.declare_dram_parameter("dbg_t5", [J, VC], F32, isOutput=True)
        dbg_sxg_d = nc.declare_dram_parameter("dbg_sxg", [J, S], F32, isOutput=True)
    cc_in = nc.dram_tensor("cc_in", [J, S], F32)
    cc_out = nc.dram_tensor("cc_out", [NCORES * J, S], F32, addr_space="Shared")

    def r(ap):
        return ap.bitcast(F32R)

    with tile.TileContext(nc) as tc:
        with ExitStack() as ctx:
            # ---- pools ----
            const = ctx.enter_context(tc.tile_pool(name="const", bufs=1))
            bigsc = ctx.enter_context(tc.tile_pool(name="bigsc", bufs=3))
            small = ctx.enter_context(tc.tile_pool(name="small", bufs=1))
            astr = ctx.enter_context(tc.tile_pool(name="astr", bufs=20))
            xtstr = ctx.enter_context(tc.tile_pool(name="xtstr", bufs=6))
            afctstr = ctx.enter_context(tc.tile_pool(name="afctstr", bufs=4))
            workf = ctx.enter_context(tc.tile_pool(name="workf", bufs=4))
            works = ctx.enter_context(tc.tile_pool(name="works", bufs=2))
            ys = ctx.enter_context(tc.tile_pool(name="ys", bufs=13))
            agp = ctx.enter_context(tc.tile_pool(name="agp", bufs=8))

            pd = ctx.enter_context(tc.tile_pool(name="pd", bufs=3, space="PSUM"))
            pcf = ctx.enter_context(tc.tile_pool(name="pcf", bufs=2, space="PSUM"))
            ptr = ctx.enter_context(tc.tile_pool(name="ptr", bufs=1, space="PSUM"))
            p5p = ctx.enter_context(tc.tile_pool(name="p5p", bufs=2, space="PSUM"))

            # ---- constants / resident loads ----
            xj_sb = const.tile([128, VT, J], F32R, tag="xj")
            for vt in range(VT):
                nc.sync.dma_start(xj_sb[:, vt, :], r(xj_d.ap()[vt * 128:(vt + 1) * 128, :]))

            afcacs_sb = const.tile([128, VT, S], F32R, tag="afcacs")
            for vt in range(VT):
                nc.sync.dma_start(afcacs_sb[:, vt, :],
                                  r(afcacs_d.ap()[vt * 128:(vt + 1) * 128, :]))

            # ---- early super-projection + allgather (before everything else
            # that PE can fill the collective latency with) ----
            sxgt_sb = small.tile([S, J], F32, tag="sxgt")
            for jc in range(2):
                ps = pd.tile([S, 384], F32, tag="pd")
                for vt in range(VT):
                    nc.tensor.matmul(ps[:], afcacs_sb[:, vt, :],
                                     xj_sb[:, vt, jc * 384:(jc + 1) * 384],
                                     start=(vt == 0), stop=(vt == VT - 1))
                nc.vector.tensor_copy(sxgt_sb[:, jc * 384:(jc + 1) * 384], ps[:])

            ident = const.tile([128, 128], F32, tag="ident")
            make_identity(nc, ident[:])

            sxg_sb = small.tile([128, JT, S], F32, tag="sxg")
            for jt in range(JT):
                tp = ptr.tile([128, S], F32, tag="ptr")
                nc.tensor.transpose(tp[:128, :], sxgt_sb[:, jt * 128:(jt + 1) * 128],
                                    ident[:S, :S])
                nc.vector.tensor_copy(sxg_sb[:, jt, :], tp[:128, :])
                nc.sync.dma_start(cc_in.ap()[jt * 128:(jt + 1) * 128, :], sxg_sb[:, jt, :])
            nc.gpsimd.collective_compute(
                "AllGather", mybir.AluOpType.bypass,
                replica_groups=[list(range(NCORES))],
                ins=[cc_in.ap()], outs=[cc_out.ap()],
            )

            afc_sb = const.tile([128, VT, VC], F32R, tag="afc")
            for vt in range(VT):
                nc.sync.dma_start(afc_sb[:, vt, :], r(afc_d.ap()[vt * 128:(vt + 1) * 128, :]))

            acs_r = const.tile([128, WCT, S], F32R, tag="acs_r")
            acs32 = const.tile([128, WCT, S], F32, tag="acs32")
            for wct in range(WCT):
                nc.sync.dma_start(acs_r[:, wct, :], r(acs_d.ap()[wct * 128:(wct + 1) * 128, :]))
                nc.sync.dma_start(acs32[:, wct, :], acs_d.ap()[wct * 128:(wct + 1) * 128, :])
            acst32 = const.tile([S, VC], F32, tag="acst32")
            nc.sync.dma_start(acst32[:], acst_d.ap())

            supc = const.tile([128, WCT, 2, VC], F32R, tag="supc")
            for k, cd in enumerate((c0_d, c1_d)):
                for wct in range(WCT):
                    nc.sync.dma_start(supc[:, wct, k, :], r(cd.ap()[wct * 128:(wct + 1) * 128, :]))

            wsbf = const.tile([128, 96], dt.bfloat16, tag="wsbf")
            ws_f32tmp = const.tile([128, 96], F32, tag="ws_f32tmp")
            nc.sync.dma_start(ws_f32tmp[:], ws_d.ap())
            nc.vector.tensor_copy(wsbf[:], ws_f32tmp[:])
            ws32 = const.tile([128, 96], F32, tag="ws32")
            nc.sync.dma_start(ws32[:], ws_d.ap())
            b128 = const.tile([128, 1], F32, tag="b128")
            nc.sync.dma_start(b128[:], b128_d.ap())
            a2i_sb = const.tile([128, AT], dt.int32, tag="a2i")
            nc.sync.dma_start(a2i_sb[:], a2i_d.ap())
            neg_half = const.tile([128, 1], F32, tag="neg_half")
            nc.gpsimd.memset(neg_half[:], -0.5)
            sc_n1 = const.tile([128, 1], F32, tag="sc_n1")
            nc.gpsimd.memset(sc_n1[:], N1)
            sc_02 = const.tile([128, 1], F32, tag="sc_02")
            nc.gpsimd.memset(sc_02[:], N2)

            # ---- phase A: xc, xcT, sxg, sxgT ----
            xc_sb = bigsc.tile([128, JT, VC], F32, tag="bigsc")
            for jt in range(JT):
                ps = pd.tile([128, VC], F32, tag="pd")
                for vt in range(VT):
                    nc.tensor.matmul(ps[:], xj_sb[:, vt, jt * 128:(jt + 1) * 128],
                                     afc_sb[:, vt, :],
                                     start=(vt == 0), stop=(vt == VT - 1))
                nc.vector.tensor_copy(xc_sb[:, jt, :], ps[:])

            xct_sb = bigsc.tile([128, WCT, J], F32R, tag="bigsc")
            for wct in range(WCT):
                for jc in range(2):
                    ps = pd.tile([128, 384], F32, tag="pd")
                    for vt in range(VT):
                        nc.tensor.matmul(
                            ps[:], afc_sb[:, vt, wct * 128:(wct + 1) * 128],
                            xj_sb[:, vt, jc * 384:(jc + 1) * 384],
                            start=(vt == 0), stop=(vt == VT - 1))
                    nc.vector.tensor_copy(xct_sb[:, wct, jc * 384:(jc + 1) * 384], ps[:])

            # ---- as_mat = P1(SXG).T @ P2(SXG), enumerated in SXG-row order ----
            as_ps = pd.tile([S, S], F32, tag="pd")
            for t in range(AT):
                a1t = agp.tile([128, S], F32, tag="a1t")
                nc.sync.dma_start(a1t[:], cc_out.ap()[t * 128:(t + 1) * 128, :])
                a2t = agp.tile([128, S], F32, tag="a2t")
                nc.gpsimd.indirect_dma_start(
                    out=a2t[:], out_offset=None,
                    in_=cc_out.ap(),
                    in_offset=bass.IndirectOffsetOnAxis(ap=a2i_sb[:, t:t + 1], axis=0),
                )
                nc.tensor.matmul(as_ps[:], a1t[:], a2t[:],
                                 start=(t == 0), stop=(t == AT - 1))

            # relu(as_mat - 0.5), then transpose
            asm = small.tile([S, S], F32, tag="asm")
            nc.scalar.activation(asm[:], as_ps[:],
                                 mybir.ActivationFunctionType.Relu, bias=neg_half[:S, :])
            asmt_ps = ptr.tile([S, S], F32, tag="ptr")
            nc.tensor.transpose(asmt_ps[:], asm[:], ident[:S, :S])
            asmt = small.tile([S, S], F32, tag="asmt")
            nc.vector.tensor_copy(asmt[:], asmt_ps[:])

            # asym_adj + softmax (rows = partitions)
            sups = []
            for mi, m in enumerate((asm, asmt)):
                rs = small.tile([S, 1], F32, tag=f"rs{mi}")
                nc.vector.tensor_reduce(rs[:], m[:], mybir.AxisListType.X,
                                        mybir.AluOpType.add)
                nc.vector.tensor_scalar_max(rs[:], rs[:], 1e-30)
                rinv = small.tile([S, 1], F32, tag=f"rinv{mi}")
                nc.vector.reciprocal(rinv[:], rs[:])
                mn = small.tile([S, S], F32, tag=f"mn{mi}")
                nc.vector.tensor_scalar_mul(mn[:], m[:], rinv[:])
                mx = small.tile([S, 1], F32, tag=f"mx{mi}")
                nc.vector.tensor_reduce(mx[:], mn[:], mybir.AxisListType.X,
                                        mybir.AluOpType.max)
                nmx = small.tile([S, 1], F32, tag=f"nmx{mi}")
                nc.vector.tensor_scalar_mul(nmx[:], mx[:], -1.0)
                e = small.tile([S, S], F32, tag=f"e{mi}")
                nc.scalar.activation(e[:], mn[:],
                                     mybir.ActivationFunctionType.Exp, bias=nmx[:])
                se = small.tile([S, 1], F32, tag=f"se{mi}")
                nc.vector.tensor_reduce(se[:], e[:], mybir.AxisListType.X,
                                        mybir.AluOpType.add)
                sinv = small.tile([S, 1], F32, tag=f"sinv{mi}")
                nc.vector.reciprocal(sinv[:], se[:])
                sup = small.tile([S, S], F32, tag=f"sup{mi}")
                nc.vector.tensor_scalar_mul(sup[:], e[:], sinv[:])
                sups.append(sup)
                if debug:
                    nc.sync.dma_start(dbg_asm_d.ap()[:, mi * S:(mi + 1) * S], sup[:])

            # ---- super diffusion + conv -> hs_pre ----
            ys_tiles = {}
            for k in range(2):
                for jt in range(JT):
                    ps = pd.tile([128, S], F32, tag="pd")
                    nc.tensor.matmul(ps[:], sxgt_sb[:, jt * 128:(jt + 1) * 128],
                                     sups[k][:], start=True, stop=True)
                    yt = ys.tile([128, S], F32, tag="ys")
                    nc.vector.tensor_copy(yt[:], ps[:])
                    ys_tiles[(k, jt)] = yt

            hs_pre = small.tile([128, JT, S], F32, tag="hs_pre")
            for jt in range(JT):
                ps = pcf.tile([128, S], F32, tag="pcf")
                parts = [sxg_sb[:, jt, :], ys_tiles[(0, jt)][:], ys_tiles[(1, jt)][:]]
                for p, rhs_full in enumerate(parts):
                    for g in range(4):
                        nc.tensor.matmul(
                            ps[32 * g:32 * (g + 1), :],
                            ws32[32 * g:32 * (g + 1), 32 * p:32 * (p + 1)],
                            rhs_full[32 * g:32 * (g + 1), :],
                            start=(p == 0), stop=(p == 2),
                            tile_position=(32 * g, 32 * g))
                nc.vector.tensor_scalar_add(hs_pre[:, jt, :], ps[:], b128[:])

            # ---- coarse diffusion + conv -> hc_pre ----
            yc_tiles = {}
            for jt in range(JT):
                ps = pd.tile([128, 2 * VC], F32, tag="pd")
                for wct in range(WCT):
                    nc.tensor.matmul(ps[:], xct_sb[:, wct, jt * 128:(jt + 1) * 128],
                                     supc[:, wct, :, :],
                                     start=(wct == 0), stop=(wct == WCT - 1))
                for k in range(2):
                    yt = ys.tile([128, VC], dt.bfloat16, tag="yc")
                    nc.vector.tensor_copy(yt[:], ps[:, k * VC:(k + 1) * VC])
                    yc_tiles[(k, jt)] = yt

            hc_pre = bigsc.tile([128, JT, VC], F32, tag="bigsc")
            for jt in range(JT):
                ps = pcf.tile([128, VC], F32, tag="pcf")
                parts = [xc_sb[:, jt, :], yc_tiles[(0, jt)][:], yc_tiles[(1, jt)][:]]
                for p, rhs_full in enumerate(parts):
                    wsel = ws32 if p == 0 else wsbf
                    for g in range(4):
                        nc.tensor.matmul(
                            ps[32 * g:32 * (g + 1), :],
                            wsel[32 * g:32 * (g + 1), 32 * p:32 * (p + 1)],
                            rhs_full[32 * g:32 * (g + 1), :],
                            start=(p == 0), stop=(p == 2),
                            tile_position=(32 * g, 32 * g))
                nc.vector.tensor_scalar_add(hc_pre[:, jt, :], ps[:], b128[:])

            # ---- step 3: hc1 = hc_pre + N1*relu(acs @ hs_pre) ----
            hspt = small.tile([S, J], F32, tag="hspt")
            for jt in range(JT):
                tp = ptr.tile([S, 128], F32, tag="ptr")
                nc.tensor.transpose(tp[:], hs_pre[:, jt, :], ident[:])
                nc.vector.tensor_copy(hspt[:, jt * 128:(jt + 1) * 128], tp[:])

            hc1 = bigsc.tile([128, JT, VC], F32, tag="bigsc")
            hc1t = small.tile([128, WCT, J], F32R, tag="hc1t")
            for jt in range(JT):
                ps = pcf.tile([128, VC], F32, tag="pcf")
                nc.tensor.matmul(ps[:], hspt[:, jt * 128:(jt + 1) * 128],
                                 acst32[:], start=True, stop=True)
                rl = works.tile([128, VC], F32, tag="rl3")
                nc.scalar.activation(rl[:], ps[:],
                                     mybir.ActivationFunctionType.Relu, scale=sc_n1[:])
                nc.vector.tensor_add(hc1[:, jt, :], hc_pre[:, jt, :], rl[:])
                for wct in range(WCT):
                    tp = ptr.tile([128, 128], F32, tag="ptr")
                    nc.tensor.transpose(tp[:], hc1[:, jt, wct * 128:(wct + 1) * 128],
                                        ident[:])
                    nc.vector.tensor_copy(
                        hc1t[:, wct, jt * 128:(jt + 1) * 128], tp[:])

            if debug:
                for jt in range(JT):
                    nc.sync.dma_start(dbg_hc1_d.ap()[jt * 128:(jt + 1) * 128, :], hc1[:, jt, :])
                    nc.sync.dma_start(dbg_hspre_d.ap()[jt * 128:(jt + 1) * 128, :], hs_pre[:, jt, :])
                    nc.sync.dma_start(dbg_sxg_d.ap()[jt * 128:(jt + 1) * 128, :], sxg_sb[:, jt, :])

            # ---- fine stage: diffusion + conv + step4 fusion + step5 accum ----
            acc5 = small.tile([128, JT, VC], F32, tag="acc5")
            nc.gpsimd.memset(acc5[:], 0.0)
            for w in range(NW):
                ablk = {}
                for vt in range(VT):
                    at = astr.tile([128, 2 * WCH], F32R, tag="ablk")
                    for k, ad in enumerate((a0_d, a1_d)):
                        nc.sync.dma_start(
                            at[:, k * WCH:(k + 1) * WCH],
                            r(ad.ap()[vt * 128:(vt + 1) * 128,
                                      w * WCH:(w + 1) * WCH]))
                    ablk[vt] = at
                afct_blk = {}
                for wct in range(WCT):
                    at = afctstr.tile([128, WCH], F32R, tag="afctblk")
                    nc.sync.dma_start(
                        at[:], r(afct_d.ap()[wct * 128:(wct + 1) * 128,
                                             w * WCH:(w + 1) * WCH]))
                    afct_blk[wct] = at

                for jt in range(JT):
                    xt_t = xtstr.tile([128, WCH], F32, tag="xtblk")
                    nc.sync.dma_start(
                        xt_t[:], xt_d.ap()[jt * 128:(jt + 1) * 128,
                                           w * WCH:(w + 1) * WCH])

                    # stage 1: both supports in one N=512 matmul per v-tile
                    psd = pd.tile([128, 2 * WCH], F32, tag="pd")
                    for vt in range(VT):
                        nc.tensor.matmul(psd[:], xj_sb[:, vt, jt * 128:(jt + 1) * 128],
                                         ablk[vt][:],
                                         start=(vt == 0), stop=(vt == VT - 1))
                    ycopies = []
                    for k in range(2):
                        yc = workf.tile([128, WCH], dt.bfloat16, tag="ycopy")
                        nc.vector.tensor_copy(yc[:], psd[:, k * WCH:(k + 1) * WCH])
                        ycopies.append(yc)

                    # stage 2 conv + stage 4 afc-term (separate psum slices)
                    pscf = pcf.tile([128, 2 * WCH], F32, tag="pcf")
                    psc = pscf[:, :WCH]
                    psf = pscf[:, WCH:]
                    parts = [xt_t[:], ycopies[0][:], ycopies[1][:]]
                    for p, rhs_full in enumerate(parts):
                        wsel = ws32 if p == 0 else wsbf
                        for g in range(4):
                            nc.tensor.matmul(
                                psc[32 * g:32 * (g + 1), :],
                                wsel[32 * g:32 * (g + 1), 32 * p:32 * (p + 1)],
                                rhs_full[32 * g:32 * (g + 1), :],
                                start=(p == 0), stop=(p == 2),
                                tile_position=(32 * g, 32 * g))
                    for wct in range(WCT):
                        nc.tensor.matmul(psf[:],
                                         hc1t[:, wct, jt * 128:(jt + 1) * 128],
                                         afct_blk[wct][:],
                                         start=(wct == 0), stop=(wct == WCT - 1))

                    rl = workf.tile([128, WCH], F32, tag="rl4")
                    nc.scalar.activation(rl[:], psf[:],
                                         mybir.ActivationFunctionType.Relu, scale=sc_02[:])
                    hft = workf.tile([128, WCH], F32, tag="hfres")
                    nc.vector.tensor_add(hft[:], psc[:], rl[:])
                    nc.vector.tensor_scalar_add(hft[:], hft[:], b128[:])
                    nc.sync.dma_start(
                        hf_d.ap()[jt * 128:(jt + 1) * 128, w * WCH:(w + 1) * WCH],
                        hft[:])

                    # step 5 accumulation: transpose hf tile, multiply by afc,
                    # accumulate into SBUF via DVE (PSUM start= clears are
                    # bank-granular, so a persistent multi-slice PSUM
                    # accumulator is unsafe)
                    ps5t = p5p.tile([128, VC], F32, tag="p5")
                    for sub in range(WCH // 128):
                        wt = w * (WCH // 128) + sub
                        tp = ptr.tile([128, 128], F32, tag="ptr")
                        nc.tensor.transpose(
                            tp[:], hft[:, sub * 128:(sub + 1) * 128], ident[:])
                        hftr = workf.tile([128, 128], F32R, tag="hftr")
                        nc.vector.tensor_copy(hftr[:], tp[:])
                        nc.tensor.matmul(ps5t[:], hftr[:],
                                         afc_sb[:, wt, :],
                                         start=(sub == 0), stop=(sub == 1))
                    nc.vector.tensor_add(acc5[:, jt, :], acc5[:, jt, :], ps5t[:])

            # ---- step 5 finish: hc2 = hc1 + N3*relu(ps5); output hc ----
            hc2 = bigsc.tile([128, JT, VC], F32, tag="bigsc")
            for jt in range(JT):
                rl = works.tile([128, VC], F32, tag="rl5")
                nc.scalar.activation(rl[:], acc5[:, jt, :],
                                     mybir.ActivationFunctionType.Relu, scale=sc_02[:])
                nc.vector.tensor_add(hc2[:, jt, :], hc1[:, jt, :], rl[:])
                if debug:
                    nc.sync.dma_start(dbg_t5_d.ap()[jt * 128:(jt + 1) * 128, :], rl[:])
                nc.sync.dma_start(hc_d.ap()[jt * 128:(jt + 1) * 128, :], hc2[:, jt, :])

            # ---- step 6: hs_out = hs_pre + N4*relu(acs.T @ hc2) ----
            for jt in range(JT):
                ps = pcf.tile([128, S], F32, tag="pcf")
                for wct in range(WCT):
                    tp = ptr.tile([128, 128], F32, tag="ptr")
                    nc.tensor.transpose(tp[:], hc2[:, jt, wct * 128:(wct + 1) * 128],
                                        ident[:])
                    h2t = works.tile([128, 128], F32, tag="h2t")
                    nc.vector.tensor_copy(h2t[:], tp[:])
                    nc.tensor.matmul(ps[:], h2t[:], acs32[:, wct, :],
                                     start=(wct == 0), stop=(wct == WCT - 1))
                rl = works.tile([128, S], F32, tag="rl6")
                nc.scalar.activation(rl[:], ps[:],
                                     mybir.ActivationFunctionType.Relu, scale=sc_02[:])
                hso = works.tile([128, S], F32, tag="hso")
                nc.vector.tensor_add(hso[:], hs_pre[:, jt, :], rl[:])
                nc.sync.dma_start(hs_d.ap()[jt * 128:(jt + 1) * 128, :], hso[:])

    nc.compile()
    return nc


def _a2_indices() -> np.ndarray:
    """Row gather indices: position enumerated by left row Lr (SXG row order);
    right row = lmajor-unflatten of the cmajor position index."""
    Lr = np.arange(NCORES * J)
    n = Lr // (L * C)
    l = (Lr // C) % L
    c = Lr % C
    i = c * (N * L) + n * L + l          # cmajor position of this left row
    l2 = i // (N * C)
    n2 = (i // C) % N
    c2 = i % C
    Rr = n2 * (L * C) + l2 * C + c2
    return Rr.astype(np.int32).reshape(AT, 128).T.copy()  # [128, AT]


def kernel(x, support, support_c, acs, afc, W, b):
    x = np.asarray(x, np.float32)
    support = np.asarray(support, np.float32)
    support_c = np.asarray(support_c, np.float32)
    acs = np.asarray(acs, np.float32)
    afc = np.asarray(afc, np.float32)
    W = np.asarray(W, np.float32)
    b = np.asarray(b, np.float32)

    if "nc" not in _CACHE:
        _CACHE["nc"] = _build()
    nc = _CACHE["nc"]

    wstack = np.zeros((128, 96), np.float32)
    for g in range(4):
        for p in range(3):
            wstack[32 * g:32 * (g + 1), 32 * p:32 * (p + 1)] = W[:, 32 * p:32 * (p + 1)].T
    b128 = b[np.arange(128) % 32].reshape(128, 1).astype(np.float32)
    a2idx = _a2_indices()

    shared = {
        "a0": np.ascontiguousarray(support[0]),
        "a1": np.ascontiguousarray(support[1]),
        "c0": np.ascontiguousarray(support_c[0]),
        "c1": np.ascontiguousarray(support_c[1]),
        "afc": afc,
        "afct": np.ascontiguousarray(afc.T),
        "acs": acs,
        "acst": np.ascontiguousarray(acs.T),
        "wstack": wstack,
        "b128": b128,
        "a2idx": a2idx,
        "afcacs": (afc.astype(np.float64) @ acs.astype(np.float64)).astype(np.float32),
    }
    in_maps = []
    for i in range(NCORES):
        xs = x[NLOC * i:NLOC * (i + 1)]
        in_maps.append(dict(
            shared,
            xj=np.ascontiguousarray(xs.transpose(2, 0, 3, 1).reshape(V, J)),
            xt=np.ascontiguousarray(xs.transpose(0, 3, 1, 2).reshape(J, V)),
        ))

    trace = bool(os.environ.get("HGCN_TRACE"))
    if trace:
        try:
            import ntff_shim  # noqa: F401
        except Exception:
            pass
    res = run_bass_kernel_spmd(nc, in_maps, list(range(NCORES)), trace=trace)
    _CACHE["last_result"] = res

    hf = np.empty((N, CO, V, L), np.float32)
    hc = np.empty((N, CO, VC, L), np.float32)
    hs = np.empty((N, CO, S, L), np.float32)
    for i in range(NCORES):
        sl = slice(NLOC * i, NLOC * (i + 1))
        hf[sl] = res.results[i]["hf"].reshape(NLOC, L, CO, V).transpose(0, 2, 3, 1)
        hc[sl] = res.results[i]["hc"].reshape(NLOC, L, CO, VC).transpose(0, 2, 3, 1)
        hs[sl] = res.results[i]["hs"].reshape(NLOC, L, CO, S).transpose(0, 2, 3, 1)
    return hf, hc, hs


# revision 14
# speedup vs baseline: 1.0938x; 1.0073x over previous
"""HGCN Bass/Trainium2 kernel — 8-core SPMD, data-parallel over batch N.

Strategy:
  - Shard batch N=16 over 8 cores (2 per core); replicate adjacencies + weights.
  - Per-core layout: j = (n, l, c) flattened (row = n*384 + l*32 + c), J = 768.
    Fine diffusion runs as out[j, w] = sum_v xj[v, j] * A[v, w] with xj resident
    in SBUF as the stationary operand (fp32r) and support streamed from HBM.
  - 1x1 conv (contraction over channels c, which live inside the partition dim)
    uses 4-group diagonal tile_position matmuls with a stacked weight tile.
  - The super-adjacency as_mat couples all batches (mismatched torch flatten
    orders), so sxg is AllGathered (196KB/rank) and as_mat computed redundantly
    per core; the right operand's row permutation is applied with an
    indirect-DMA gather using host-precomputed indices.
  - Hierarchical relu fusions: afc/acs contractions via PE with on-the-fly PE
    transposes; step-5 (afc.T @ hf) accumulates into a persistent 3-bank PSUM
    tensor while hf tiles stream out.
"""
import os
import sys

sys.path.insert(0, "/opt/trn_rl_repo")

import numpy as np
from contextlib import ExitStack

import concourse.bass as bass
import concourse.tile as tile
from concourse import bacc, mybir
from concourse.bass_utils import run_bass_kernel_spmd
from concourse.masks import make_identity

dt = mybir.dt
F32 = dt.float32
F32R = dt.float32r

# dims
N, C, V, VC, S, L, CO = 16, 32, 2048, 256, 64, 12, 32
N1, N2, N3, N4 = 0.8, 0.2, 0.2, 0.2
NCORES = 8
NLOC = N // NCORES            # 2 batches per core
J = NLOC * L * C              # 768 local (n,l,c) columns
JT = J // 128                 # 6 j-tiles
VT = V // 128                 # 16 v-tiles
WCH = 256                     # fine w chunk (free dim)
NW = V // WCH                 # 8 w chunks
WCT = VC // 128               # 2 coarse-node tiles
AT = (NCORES * J) // 128      # 48 as_mat contraction tiles

_CACHE = {}


def _build():
    from concourse.compiler_utils import get_compiler_flags, set_compiler_flags
    set_compiler_flags([f.replace("--enable-ldw-opt=false", "--enable-ldw-opt=true")
                        for f in get_compiler_flags()])
    nc = bacc.Bacc("TRN2", target_bir_lowering=False, debug=False,
                   num_devices=NCORES)

    # ---- I/O ----
    xj_d = nc.declare_dram_parameter("xj", [V, J], F32, isOutput=False)
    xt_d = nc.declare_dram_parameter("xt", [J, V], F32, isOutput=False)
    a0_d = nc.declare_dram_parameter("a0", [V, V], F32, isOutput=False)
    a1_d = nc.declare_dram_parameter("a1", [V, V], F32, isOutput=False)
    c0_d = nc.declare_dram_parameter("c0", [VC, VC], F32, isOutput=False)
    c1_d = nc.declare_dram_parameter("c1", [VC, VC], F32, isOutput=False)
    afc_d = nc.declare_dram_parameter("afc", [V, VC], F32, isOutput=False)
    afct_d = nc.declare_dram_parameter("afct", [VC, V], F32, isOutput=False)
    acs_d = nc.declare_dram_parameter("acs", [VC, S], F32, isOutput=False)
    acst_d = nc.declare_dram_parameter("acst", [S, VC], F32, isOutput=False)
    ws_d = nc.declare_dram_parameter("wstack", [128, 96], F32, isOutput=False)
    b128_d = nc.declare_dram_parameter("b128", [128, 1], F32, isOutput=False)
    a2i_d = nc.declare_dram_parameter("a2idx", [128, AT], dt.int32, isOutput=False)
    afcacs_d = nc.declare_dram_parameter("afcacs", [V, S], F32, isOutput=False)

    debug = bool(os.environ.get("HGCN_DEBUG"))
    hf_d = nc.declare_dram_parameter("hf", [J, V], F32, isOutput=True)
    hc_d = nc.declare_dram_parameter("hc", [J, VC], F32, isOutput=True)
    hs_d = nc.declare_dram_parameter("hs", [J, S], F32, isOutput=True)

    if debug:
        dbg_asm_d = nc.declare_dram_parameter("dbg_asm", [S, 2 * S], F32, isOutput=True)
        dbg_hc1_d = nc.declare_dram_parameter("dbg_hc1", [J, VC], F32, isOutput=True)
        dbg_hspre_d = nc.declare_dram_parameter("dbg_hspre", [J, S], F32, isOutput=True)
        dbg_t5_d = nc.declare_dram_parameter("dbg_t5", [J, VC], F32, isOutput=True)
        dbg_sxg_d = nc.declare_dram_parameter("dbg_sxg", [J, S], F32, isOutput=True)
    cc_in = nc.dram_tensor("cc_in", [J, S], F32)
    cc_out = nc.dram_tensor("cc_out", [NCORES * J, S], F32, addr_space="Shared")

    def r(ap):
        return ap.bitcast(F32R)

    with tile.TileContext(nc) as tc:
        with ExitStack() as ctx:
            # ---- pools ----
            const = ctx.enter_context(tc.tile_pool(name="const", bufs=1))
            bigsc = ctx.enter_context(tc.tile_pool(name="bigsc", bufs=3))
            small = ctx.enter_context(tc.tile_pool(name="small", bufs=1))
            astr = ctx.enter_context(tc.tile_pool(name="astr", bufs=20))
            xtstr = ctx.enter_context(tc.tile_pool(name="xtstr", bufs=4))
            afctstr = ctx.enter_context(tc.tile_pool(name="afctstr", bufs=4))
            workf = ctx.enter_context(tc.tile_pool(name="workf", bufs=4))
            works = ctx.enter_context(tc.tile_pool(name="works", bufs=2))
            ys = ctx.enter_context(tc.tile_pool(name="ys", bufs=12))
            agp = ctx.enter_context(tc.tile_pool(name="agp", bufs=4))

            pd = ctx.enter_context(tc.tile_pool(name="pd", bufs=3, space="PSUM"))
            pcf = ctx.enter_context(tc.tile_pool(name="pcf", bufs=2, space="PSUM"))
            ptr = ctx.enter_context(tc.tile_pool(name="ptr", bufs=1, space="PSUM"))
            p5p = ctx.enter_context(tc.tile_pool(name="p5p", bufs=2, space="PSUM"))

            # ---- constants / resident loads ----
            xj_sb = const.tile([128, VT, J], F32R, tag="xj")
            for vt in range(VT):
                nc.sync.dma_start(xj_sb[:, vt, :], r(xj_d.ap()[vt * 128:(vt + 1) * 128, :]))

            afcacs_sb = const.tile([128, VT, S], F32R, tag="afcacs")
            for vt in range(VT):
                nc.sync.dma_start(afcacs_sb[:, vt, :],
                                  r(afcacs_d.ap()[vt * 128:(vt + 1) * 128, :]))

            wsbf = const.tile([128, 96], dt.bfloat16, tag="wsbf")
            ws_f32tmp = const.tile([128, 96], F32, tag="ws_f32tmp")
            nc.sync.dma_start(ws_f32tmp[:], ws_d.ap())
            nc.vector.tensor_copy(wsbf[:], ws_f32tmp[:])
            ws32 = const.tile([128, 96], F32, tag="ws32")
            nc.sync.dma_start(ws32[:], ws_d.ap())
            b128 = const.tile([128, 1], F32, tag="b128")
            nc.sync.dma_start(b128[:], b128_d.ap())
            acc5 = small.tile([128, JT, VC], F32, tag="acc5")
            nc.gpsimd.memset(acc5[:], 0.0)
            hfc = small.tile([128, 2, JT, WCH], F32, tag="hfc")

            # ---- early super-projection + allgather (before everything else
            # that PE can fill the collective latency with) ----
            sxgt_sb = small.tile([S, J], F32, tag="sxgt")
            for jc in range(2):
                ps = pd.tile([S, 384], F32, tag="pd")
                for vt in range(VT):
                    nc.tensor.matmul(ps[:], afcacs_sb[:, vt, :],
                                     xj_sb[:, vt, jc * 384:(jc + 1) * 384],
                                     start=(vt == 0), stop=(vt == VT - 1))
                nc.vector.tensor_copy(sxgt_sb[:, jc * 384:(jc + 1) * 384], ps[:])

            ident = const.tile([128, 128], F32, tag="ident")
            make_identity(nc, ident[:])

            sxg_sb = small.tile([128, JT, S], F32, tag="sxg")
            for jt in range(JT):
                tp = ptr.tile([128, S], F32, tag="ptr")
                nc.tensor.transpose(tp[:128, :], sxgt_sb[:, jt * 128:(jt + 1) * 128],
                                    ident[:S, :S])
                nc.vector.tensor_copy(sxg_sb[:, jt, :], tp[:128, :])
                nc.sync.dma_start(cc_in.ap()[jt * 128:(jt + 1) * 128, :], sxg_sb[:, jt, :])
            nc.gpsimd.collective_compute(
                "AllGather", mybir.AluOpType.bypass,
                replica_groups=[list(range(NCORES))],
                ins=[cc_in.ap()], outs=[cc_out.ap()],
            )

            def fine_stage12(w, jt, ablk):
                xt_t = xtstr.tile([128, WCH], F32, tag="xtblk")
                nc.sync.dma_start(
                    xt_t[:], xt_d.ap()[jt * 128:(jt + 1) * 128,
                                       w * WCH:(w + 1) * WCH])
                psd = pd.tile([128, 2 * WCH], F32, tag="pd")
                for vt in range(VT):
                    nc.tensor.matmul(psd[:], xj_sb[:, vt, jt * 128:(jt + 1) * 128],
                                     ablk[vt][:],
                                     start=(vt == 0), stop=(vt == VT - 1))
                ycopies = []
                for k in range(2):
                    yc = workf.tile([128, WCH], dt.bfloat16, tag="ycopy")
                    nc.vector.tensor_copy(yc[:], psd[:, k * WCH:(k + 1) * WCH])
                    ycopies.append(yc)
                parts = [xt_t[:], ycopies[0][:], ycopies[1][:]]
                return parts

            def conv_into(psc, parts, w, jt):
                for p, rhs_full in enumerate(parts):
                    wsel = ws32 if p == 0 else wsbf
                    for g in range(4):
                        nc.tensor.matmul(
                            psc[32 * g:32 * (g + 1), :],
                            wsel[32 * g:32 * (g + 1), 32 * p:32 * (p + 1)],
                            rhs_full[32 * g:32 * (g + 1), :],
                            start=(p == 0), stop=(p == 2),
                            tile_position=(32 * g, 32 * g))

            def load_ablk(w):
                ablk = {}
                for vt in range(VT):
                    at = astr.tile([128, 2 * WCH], F32R, tag="ablk")
                    for k, ad in enumerate((a0_d, a1_d)):
                        nc.sync.dma_start(
                            at[:, k * WCH:(k + 1) * WCH],
                            r(ad.ap()[vt * 128:(vt + 1) * 128,
                                      w * WCH:(w + 1) * WCH]))
                    ablk[vt] = at
                return ablk

            def load_afct(w):
                afct_blk = {}
                for wct in range(WCT):
                    at = afctstr.tile([128, WCH], F32R, tag="afctblk")
                    nc.sync.dma_start(
                        at[:], r(afct_d.ap()[wct * 128:(wct + 1) * 128,
                                             w * WCH:(w + 1) * WCH]))
                    afct_blk[wct] = at
                return afct_blk

            def fine_tail(w, jt, psf, base_sbuf):
                # base_sbuf: None -> psum psc path handled by caller via hft add
                rl = workf.tile([128, WCH], F32, tag="rl4")
                nc.scalar.activation(rl[:], psf[:],
                                     mybir.ActivationFunctionType.Relu, scale=sc_02[:])
                hft = workf.tile([128, WCH], F32, tag="hfres")
                nc.vector.tensor_add(hft[:], base_sbuf, rl[:])
                nc.vector.tensor_scalar_add(hft[:], hft[:], b128[:])
                nc.sync.dma_start(
                    hf_d.ap()[jt * 128:(jt + 1) * 128, w * WCH:(w + 1) * WCH],
                    hft[:])
                ps5t = p5p.tile([128, VC], F32, tag="p5")
                for sub in range(WCH // 128):
                    wt = w * (WCH // 128) + sub
                    tp = ptr.tile([128, 128], F32, tag="ptr")
                    nc.tensor.transpose(
                        tp[:], hft[:, sub * 128:(sub + 1) * 128], ident[:])
                    hftr = workf.tile([128, 128], F32R, tag="hftr")
                    nc.vector.tensor_copy(hftr[:], tp[:])
                    nc.tensor.matmul(ps5t[:], hftr[:],
                                     afc_sb[:, wt, :],
                                     start=(sub == 0), stop=(sub == 1))
                nc.vector.tensor_add(acc5[:, jt, :], acc5[:, jt, :], ps5t[:])

            # deferred pass: stage 1+2 for w=0,1 runs here (before the
            # allgather-dependent chain in PE program order) to hide the
            # collective + as_mat latency; stage 4 catches up later.
            for w in range(2):
                ablk = load_ablk(w)
                for jt in range(JT):
                    parts = fine_stage12(w, jt, ablk)
                    pscd = pcf.tile([128, WCH], F32, tag="pcf")
                    conv_into(pscd[:, :WCH], parts, w, jt)
                    nc.vector.tensor_copy(hfc[:, w, jt, :], pscd[:, :WCH])


            afc_sb = const.tile([128, VT, VC], F32R, tag="afc")
            for vt in range(VT):
                nc.sync.dma_start(afc_sb[:, vt, :], r(afc_d.ap()[vt * 128:(vt + 1) * 128, :]))

            acs_r = const.tile([128, WCT, S], F32R, tag="acs_r")
            acs32 = const.tile([128, WCT, S], F32, tag="acs32")
            for wct in range(WCT):
                nc.sync.dma_start(acs_r[:, wct, :], r(acs_d.ap()[wct * 128:(wct + 1) * 128, :]))
                nc.sync.dma_start(acs32[:, wct, :], acs_d.ap()[wct * 128:(wct + 1) * 128, :])
            acst32 = const.tile([S, VC], F32, tag="acst32")
            nc.sync.dma_start(acst32[:], acst_d.ap())

            supc = const.tile([128, WCT, 2, VC], F32R, tag="supc")
            for k, cd in enumerate((c0_d, c1_d)):
                for wct in range(WCT):
                    nc.sync.dma_start(supc[:, wct, k, :], r(cd.ap()[wct * 128:(wct + 1) * 128, :]))

            a2i_sb = const.tile([128, AT], dt.int32, tag="a2i")
            nc.sync.dma_start(a2i_sb[:], a2i_d.ap())
            neg_half = const.tile([128, 1], F32, tag="neg_half")
            nc.gpsimd.memset(neg_half[:], -0.5)
            sc_n1 = const.tile([128, 1], F32, tag="sc_n1")
            nc.gpsimd.memset(sc_n1[:], N1)
            sc_02 = const.tile([128, 1], F32, tag="sc_02")
            nc.gpsimd.memset(sc_02[:], N2)

            # ---- phase A: xc, xcT, sxg, sxgT ----
            xc_sb = bigsc.tile([128, JT, VC], F32, tag="bigsc")
            for jt in range(JT):
                ps = pd.tile([128, VC], F32, tag="pd")
                for vt in range(VT):
                    nc.tensor.matmul(ps[:], xj_sb[:, vt, jt * 128:(jt + 1) * 128],
                                     afc_sb[:, vt, :],
                                     start=(vt == 0), stop=(vt == VT - 1))
                nc.vector.tensor_copy(xc_sb[:, jt, :], ps[:])

            xct_sb = bigsc.tile([128, WCT, J], F32R, tag="bigsc")
            for wct in range(WCT):
                for jc in range(2):
                    ps = pd.tile([128, 384], F32, tag="pd")
                    for vt in range(VT):
                        nc.tensor.matmul(
                            ps[:], afc_sb[:, vt, wct * 128:(wct + 1) * 128],
                            xj_sb[:, vt, jc * 384:(jc + 1) * 384],
                            start=(vt == 0), stop=(vt == VT - 1))
                    nc.vector.tensor_copy(xct_sb[:, wct, jc * 384:(jc + 1) * 384], ps[:])

            # ---- as_mat = P1(SXG).T @ P2(SXG), enumerated in SXG-row order ----
            as_ps = pd.tile([S, S], F32, tag="pd")
            for t in range(AT):
                a1t = agp.tile([128, S], F32, tag="a1t")
                nc.sync.dma_start(a1t[:], cc_out.ap()[t * 128:(t + 1) * 128, :])
                a2t = agp.tile([128, S], F32, tag="a2t")
                nc.gpsimd.indirect_dma_start(
                    out=a2t[:], out_offset=None,
                    in_=cc_out.ap(),
                    in_offset=bass.IndirectOffsetOnAxis(ap=a2i_sb[:, t:t + 1], axis=0),
                )
                nc.tensor.matmul(as_ps[:], a1t[:], a2t[:],
                                 start=(t == 0), stop=(t == AT - 1))

            # relu(as_mat - 0.5), then transpose
            asm = small.tile([S, S], F32, tag="asm")
            nc.scalar.activation(asm[:], as_ps[:],
                                 mybir.ActivationFunctionType.Relu, bias=neg_half[:S, :])
            asmt_ps = ptr.tile([S, S], F32, tag="ptr")
            nc.tensor.transpose(asmt_ps[:], asm[:], ident[:S, :S])
            asmt = small.tile([S, S], F32, tag="asmt")
            nc.vector.tensor_copy(asmt[:], asmt_ps[:])

            # asym_adj + softmax (rows = partitions)
            sups = []
            for mi, m in enumerate((asm, asmt)):
                rs = small.tile([S, 1], F32, tag=f"rs{mi}")
                nc.vector.tensor_reduce(rs[:], m[:], mybir.AxisListType.X,
                                        mybir.AluOpType.add)
                nc.vector.tensor_scalar_max(rs[:], rs[:], 1e-30)
                rinv = small.tile([S, 1], F32, tag=f"rinv{mi}")
                nc.vector.reciprocal(rinv[:], rs[:])
                mn = small.tile([S, S], F32, tag=f"mn{mi}")
                nc.vector.tensor_scalar_mul(mn[:], m[:], rinv[:])
                mx = small.tile([S, 1], F32, tag=f"mx{mi}")
                nc.vector.tensor_reduce(mx[:], mn[:], mybir.AxisListType.X,
                                        mybir.AluOpType.max)
                nmx = small.tile([S, 1], F32, tag=f"nmx{mi}")
                nc.vector.tensor_scalar_mul(nmx[:], mx[:], -1.0)
                e = small.tile([S, S], F32, tag=f"e{mi}")
                nc.scalar.activation(e[:], mn[:],
                                     mybir.ActivationFunctionType.Exp, bias=nmx[:])
                se = small.tile([S, 1], F32, tag=f"se{mi}")
                nc.vector.tensor_reduce(se[:], e[:], mybir.AxisListType.X,
                                        mybir.AluOpType.add)
                sinv = small.tile([S, 1], F32, tag=f"sinv{mi}")
                nc.vector.reciprocal(sinv[:], se[:])
                sup = small.tile([S, S], F32, tag=f"sup{mi}")
                nc.vector.tensor_scalar_mul(sup[:], e[:], sinv[:])
                sups.append(sup)
                if debug:
                    nc.sync.dma_start(dbg_asm_d.ap()[:, mi * S:(mi + 1) * S], sup[:])

            # ---- super diffusion + conv -> hs_pre ----
            ys_tiles = {}
            for k in range(2):
                for jt in range(JT):
                    ps = pd.tile([128, S], F32, tag="pd")
                    nc.tensor.matmul(ps[:], sxgt_sb[:, jt * 128:(jt + 1) * 128],
                                     sups[k][:], start=True, stop=True)
                    yt = ys.tile([128, S], F32, tag="ys")
                    nc.vector.tensor_copy(yt[:], ps[:])
                    ys_tiles[(k, jt)] = yt

            hs_pre = small.tile([128, JT, S], F32, tag="hs_pre")
            for jt in range(JT):
                ps = pcf.tile([128, S], F32, tag="pcf")
                parts = [sxg_sb[:, jt, :], ys_tiles[(0, jt)][:], ys_tiles[(1, jt)][:]]
                for p, rhs_full in enumerate(parts):
                    for g in range(4):
                        nc.tensor.matmul(
                            ps[32 * g:32 * (g + 1), :],
                            ws32[32 * g:32 * (g + 1), 32 * p:32 * (p + 1)],
                            rhs_full[32 * g:32 * (g + 1), :],
                            start=(p == 0), stop=(p == 2),
                            tile_position=(32 * g, 32 * g))
                nc.vector.tensor_scalar_add(hs_pre[:, jt, :], ps[:], b128[:])

            # ---- coarse diffusion + conv -> hc_pre ----
            yc_tiles = {}
            for jt in range(JT):
                ps = pd.tile([128, 2 * VC], F32, tag="pd")
                for wct in range(WCT):
                    nc.tensor.matmul(ps[:], xct_sb[:, wct, jt * 128:(jt + 1) * 128],
                                     supc[:, wct, :, :],
                                     start=(wct == 0), stop=(wct == WCT - 1))
                for k in range(2):
                    yt = ys.tile([128, VC], dt.bfloat16, tag="yc")
                    nc.vector.tensor_copy(yt[:], ps[:, k * VC:(k + 1) * VC])
                    yc_tiles[(k, jt)] = yt

            hc_pre = bigsc.tile([128, JT, VC], F32, tag="bigsc")
            for jt in range(JT):
                ps = pcf.tile([128, VC], F32, tag="pcf")
                parts = [xc_sb[:, jt, :], yc_tiles[(0, jt)][:], yc_tiles[(1, jt)][:]]
                for p, rhs_full in enumerate(parts):
                    wsel = ws32 if p == 0 else wsbf
                    for g in range(4):
                        nc.tensor.matmul(
                            ps[32 * g:32 * (g + 1), :],
                            wsel[32 * g:32 * (g + 1), 32 * p:32 * (p + 1)],
                            rhs_full[32 * g:32 * (g + 1), :],
                            start=(p == 0), stop=(p == 2),
                            tile_position=(32 * g, 32 * g))
                nc.vector.tensor_scalar_add(hc_pre[:, jt, :], ps[:], b128[:])

            # ---- step 3: hc1 = hc_pre + N1*relu(acs @ hs_pre) ----
            hspt = small.tile([S, J], F32, tag="hspt")
            for jt in range(JT):
                tp = ptr.tile([S, 128], F32, tag="ptr")
                nc.tensor.transpose(tp[:], hs_pre[:, jt, :], ident[:])
                nc.vector.tensor_copy(hspt[:, jt * 128:(jt + 1) * 128], tp[:])

            hc1 = bigsc.tile([128, JT, VC], F32, tag="bigsc")
            hc1t = small.tile([128, WCT, J], F32R, tag="hc1t")
            for jt in range(JT):
                ps = pcf.tile([128, VC], F32, tag="pcf")
                nc.tensor.matmul(ps[:], hspt[:, jt * 128:(jt + 1) * 128],
                                 acst32[:], start=True, stop=True)
                rl = works.tile([128, VC], F32, tag="rl3")
                nc.scalar.activation(rl[:], ps[:],
                                     mybir.ActivationFunctionType.Relu, scale=sc_n1[:])
                nc.vector.tensor_add(hc1[:, jt, :], hc_pre[:, jt, :], rl[:])
                for wct in range(WCT):
                    tp = ptr.tile([128, 128], F32, tag="ptr")
                    nc.tensor.transpose(tp[:], hc1[:, jt, wct * 128:(wct + 1) * 128],
                                        ident[:])
                    nc.vector.tensor_copy(
                        hc1t[:, wct, jt * 128:(jt + 1) * 128], tp[:])

            if debug:
                for jt in range(JT):
                    nc.sync.dma_start(dbg_hc1_d.ap()[jt * 128:(jt + 1) * 128, :], hc1[:, jt, :])
                    nc.sync.dma_start(dbg_hspre_d.ap()[jt * 128:(jt + 1) * 128, :], hs_pre[:, jt, :])
                    nc.sync.dma_start(dbg_sxg_d.ap()[jt * 128:(jt + 1) * 128, :], sxg_sb[:, jt, :])

            # ---- fine stage: diffusion + conv + step4 fusion + step5 accum ----
            # catch-up stage 4 for the deferred w=0,1
            for w in range(2):
                afct_blk = load_afct(w)
                for jt in range(JT):
                    pscf = pcf.tile([128, WCH], F32, tag="pcf")
                    for wct in range(WCT):
                        nc.tensor.matmul(pscf[:],
                                         hc1t[:, wct, jt * 128:(jt + 1) * 128],
                                         afct_blk[wct][:],
                                         start=(wct == 0), stop=(wct == WCT - 1))
                    fine_tail(w, jt, pscf[:], hfc[:, w, jt, :])

            # fused path for remaining w-chunks
            for w in range(2, NW):
                ablk = load_ablk(w)
                afct_blk = load_afct(w)
                for jt in range(JT):
                    parts = fine_stage12(w, jt, ablk)
                    pscf = pcf.tile([128, 2 * WCH], F32, tag="pcf")
                    psc = pscf[:, :WCH]
                    psf = pscf[:, WCH:]
                    conv_into(psc, parts, w, jt)
                    for wct in range(WCT):
                        nc.tensor.matmul(psf[:],
                                         hc1t[:, wct, jt * 128:(jt + 1) * 128],
                                         afct_blk[wct][:],
                                         start=(wct == 0), stop=(wct == WCT - 1))
                    fine_tail(w, jt, psf[:], psc[:])

            # ---- step 5 finish: hc2 = hc1 + N3*relu(ps5); output hc ----
            hc2 = bigsc.tile([128, JT, VC], F32, tag="bigsc")
            for jt in range(JT):
                rl = works.tile([128, VC], F32, tag="rl5")
                nc.scalar.activation(rl[:], acc5[:, jt, :],
                                     mybir.ActivationFunctionType.Relu, scale=sc_02[:])
                nc.vector.tensor_add(hc2[:, jt, :], hc1[:, jt, :], rl[:])
                if debug:
                    nc.sync.dma_start(dbg_t5_d.ap()[jt * 128:(jt + 1) * 128, :], rl[:])
                nc.sync.dma_start(hc_d.ap()[jt * 128:(jt + 1) * 128, :], hc2[:, jt, :])

            # ---- step 6: hs_out = hs_pre + N4*relu(acs.T @ hc2) ----
            for jt in range(JT):
                ps = pcf.tile([128, S], F32, tag="pcf")
                for wct in range(WCT):
                    tp = ptr.tile([128, 128], F32, tag="ptr")
                    nc.tensor.transpose(tp[:], hc2[:, jt, wct * 128:(wct + 1) * 128],
                                        ident[:])
                    h2t = works.tile([128, 128], F32, tag="h2t")
                    nc.vector.tensor_copy(h2t[:], tp[:])
                    nc.tensor.matmul(ps[:], h2t[:], acs32[:, wct, :],
                                     start=(wct == 0), stop=(wct == WCT - 1))
                rl = works.tile([128, S], F32, tag="rl6")
                nc.scalar.activation(rl[:], ps[:],
                                     mybir.ActivationFunctionType.Relu, scale=sc_02[:])
                hso = works.tile([128, S], F32, tag="hso")
                nc.vector.tensor_add(hso[:], hs_pre[:, jt, :], rl[:])
                nc.sync.dma_start(hs_d.ap()[jt * 128:(jt + 1) * 128, :], hso[:])

    nc.compile()
    return nc


def _a2_indices() -> np.ndarray:
    """Row gather indices: position enumerated by left row Lr (SXG row order);
    right row = lmajor-unflatten of the cmajor position index."""
    Lr = np.arange(NCORES * J)
    n = Lr // (L * C)
    l = (Lr // C) % L
    c = Lr % C
    i = c * (N * L) + n * L + l          # cmajor position of this left row
    l2 = i // (N * C)
    n2 = (i // C) % N
    c2 = i % C
    Rr = n2 * (L * C) + l2 * C + c2
    return Rr.astype(np.int32).reshape(AT, 128).T.copy()  # [128, AT]


def kernel(x, support, support_c, acs, afc, W, b):
    x = np.asarray(x, np.float32)
    support = np.asarray(support, np.float32)
    support_c = np.asarray(support_c, np.float32)
    acs = np.asarray(acs, np.float32)
    afc = np.asarray(afc, np.float32)
    W = np.asarray(W, np.float32)
    b = np.asarray(b, np.float32)

    if "nc" not in _CACHE:
        _CACHE["nc"] = _build()
    nc = _CACHE["nc"]

    wstack = np.zeros((128, 96), np.float32)
    for g in range(4):
        for p in range(3):
            wstack[32 * g:32 * (g + 1), 32 * p:32 * (p + 1)] = W[:, 32 * p:32 * (p + 1)].T
    b128 = b[np.arange(128) % 32].reshape(128, 1).astype(np.float32)
    a2idx = _a2_indices()

    shared = {
        "a0": np.ascontiguousarray(support[0]),
        "a1": np.ascontiguousarray(support[1]),
        "c0": np.ascontiguousarray(support_c[0]),
        "c1": np.ascontiguousarray(support_c[1]),
        "afc": afc,
        "afct": np.ascontiguousarray(afc.T),
        "acs": acs,
        "acst": np.ascontiguousarray(acs.T),
        "wstack": wstack,
        "b128": b128,
        "a2idx": a2idx,
        "afcacs": (afc.astype(np.float64) @ acs.astype(np.float64)).astype(np.float32),
    }
    in_maps = []
    for i in range(NCORES):
        xs = x[NLOC * i:NLOC * (i + 1)]
        in_maps.append(dict(
            shared,
            xj=np.ascontiguousarray(xs.transpose(2, 0, 3, 1).reshape(V, J)),
            xt=np.ascontiguousarray(xs.transpose(0, 3, 1, 2).reshape(J, V)),
        ))

    trace = bool(os.environ.get("HGCN_TRACE"))
    if trace:
        try:
            import ntff_shim  # noqa: F401
        except Exception:
            pass
    res = run_bass_kernel_spmd(nc, in_maps, list(range(NCORES)), trace=trace)
    _CACHE["last_result"] = res

    hf = np.empty((N, CO, V, L), np.float32)
    hc = np.empty((N, CO, VC, L), np.float32)
    hs = np.empty((N, CO, S, L), np.float32)
    for i in range(NCORES):
        sl = slice(NLOC * i, NLOC * (i + 1))
        hf[sl] = res.results[i]["hf"].reshape(NLOC, L, CO, V).transpose(0, 2, 3, 1)
        hc[sl] = res.results[i]["hc"].reshape(NLOC, L, CO, VC).transpose(0, 2, 3, 1)
        hs[sl] = res.results[i]["hs"].reshape(NLOC, L, CO, S).transpose(0, 2, 3, 1)
    return hf, hc, hs
